# revision 1
# baseline (speedup 1.0000x reference)
"""Distributed Trainium2 (Bass/Tile) kernel for nn_Attention_2D.

Pipeline (per batch element): 3x3 conv + BatchNorm (batch stats!) for
Q (from x), K, V (from y) -> linear projections -> multi-head attention
(scale = C**-0.5) -> output projection.

Sharding: data-parallel over batch B=8 across the 8 NeuronCores (one
image per core). The only cross-core dependency is the BatchNorm
mean/var over the whole batch -> a tiny [128,12] AllReduce.

Device layout notes:
  - images are stored channel-major [C, L] (C on partitions, 2 chunks of
    128), so BN is a per-partition affine and conv = 9 shifted matmuls
    with weight tiles [ci, co].
  - conv inputs live in a zero-padded [c, 34, 34] buffer so the 9 shifts
    are strided access patterns of one buffer.
  - attention is computed in the transposed orientation S^T[t, l] with t
    on partitions; 4 heads run concurrently in the PE array via
    row-tiling (K=32 each).  exp() runs on ScalarE straight out of PSUM
    with the 1/16 scale folded in, writing bf16 probabilities P^T.
  - attn@V uses col-tiling (M=32 per head, 4 heads concurrent) to
    produce the output directly transposed [c, l]; an all-ones [128,32]
    stationary operand produces the softmax denominators pre-broadcast
    across each head's 32 partitions, so normalization is one
    reciprocal + one multiply.
  - matmuls use float32r (full PE speed at N>=256, fp32 storage; DMA
    into an f32r tile performs the rounding the BIR verifier requires).
"""

import numpy as np

B, L, C = 8, 1024, 256
H = 8
D = 32  # head dim
IMG = 32  # h = w = 32
PAD = 34  # padded image side
EPS = 1e-5
ATT_SCALE = float(C) ** -0.5  # 1/16

_CACHE = {}
DEBUG = False
VARIANT = "full"  # "full" | "noattn" | "convonly" (phase timing builds)


def _build_nc(repeat=1):
    import concourse.bacc as bacc
    import concourse.tile as tile
    from concourse import mybir

    f32 = mybir.dt.float32
    f32r = mybir.dt.float32r
    bf16 = mybir.dt.bfloat16
    AF = mybir.ActivationFunctionType
    ALU = mybir.AluOpType

    nc = bacc.Bacc(None, target_bir_lowering=False)
    nc.num_devices = 8

    # ---- DRAM parameters (host-prepped layouts) ----
    # x[b].T zero-padded to 34x34 (host bakes the conv padding)
    xt = nc.declare_dram_parameter("xt", [C, PAD * PAD], f32r, isOutput=False)
    yt = nc.declare_dram_parameter("yt", [C, PAD * PAD], f32r, isOutput=False)
    # conv weights: [9(kpos), 2(ci), 2(co), 128, 128] with w[kp,ci,co,p,f] =
    # conv_w[co*128+f, ci*128+p, ky, kx]
    wcq = nc.declare_dram_parameter("wcq", [9, 2, 2, 128, 128], f32r, isOutput=False)
    wck = nc.declare_dram_parameter("wck", [9, 2, 2, 128, 128], f32r, isOutput=False)
    wcv = nc.declare_dram_parameter("wcv", [9, 2, 2, 128, 128], f32r, isOutput=False)
    # projection weights W.T tiled: [2(ci), 128, 256(co)]
    pq = nc.declare_dram_parameter("pq", [2, 128, C], f32r, isOutput=False)
    pk = nc.declare_dram_parameter("pk", [2, 128, C], f32r, isOutput=False)
    pv = nc.declare_dram_parameter("pv", [2, 128, C], f32r, isOutput=False)
    po = nc.declare_dram_parameter("po", [2, 128, C], f32r, isOutput=False)
    # gamma/beta pack [128, 12]: cols 0-5 gamma, 6-11 beta, col order
    # (q c0, q c1, k c0, k c1, v c0, v c1)
    gb = nc.declare_dram_parameter("gb", [128, 12], f32, isOutput=False)
    bo = nc.declare_dram_parameter("bo", [128, 2], f32, isOutput=False)
    out = nc.declare_dram_parameter("out", [C, L], f32, isOutput=True)
    dbg = {}
    if DEBUG:
        for name, shape, dt_ in (
            ("dkraw", [128, 2 * L], f32), ("dkbn", [128, 2 * L], f32),
            ("dst", [128, 12], f32), ("dgst", [128, 12], f32),
            ("dscale", [128, 6], f32), ("dshift", [128, 6], f32),
            ("dqT", [128, 2 * L], f32), ("dkT", [128, 2 * L], f32),
            ("dvsb", [128, 8 * C], mybir.dt.bfloat16),
            ("dptt", [128, 4 * 8 * 512], mybir.dt.bfloat16),
            ("daoT", [128, 2 * L], f32),
        ):
            dbg[name] = nc.declare_dram_parameter(name, shape, dt_, isOutput=True)

    with tile.TileContext(nc) as tc:
        with tc.tile_pool(name="singles", bufs=1) as singles, \
             tc.tile_pool(name="stats", bufs=1) as statsp, \
             tc.tile_pool(name="bnst", bufs=4) as bnstp, \
             tc.tile_pool(name="rep", bufs=2) as repp, \
             tc.tile_pool(name="pt", bufs=2) as ptp, \
             tc.tile_pool(name="stg", bufs=2) as stgp, \
             tc.tile_pool(name="ps", bufs=2, space="PSUM") as psp, \
             tc.tile_pool(name="score_ps", bufs=3, space="PSUM") as scorep, \
             tc.tile_pool(name="dram", bufs=1, space="DRAM") as dramp:

            for _rep in range(repeat):
                # ---------- constants / small tiles ----------
                ones32 = singles.tile([128, 32], bf16)
                nc.vector.memset(ones32[:], 1.0)
                epst = singles.tile([128, 1], f32)
                nc.vector.memset(epst[:], EPS)
                gbt = singles.tile([128, 12], f32)
                nc.sync.dma_start(out=gbt[:], in_=gb[:])
                bot = singles.tile([128, 2], f32)
                nc.sync.dma_start(out=bot[:], in_=bo[:])

                # ---------- padded images + weights ----------
                # Two HWDGE rings run in parallel and each ring is FIFO, so
                # emit in consumption order: the q path (conv_q runs first) on
                # the sync ring, the k/v path on the scalar ring.
                pad_x = singles.tile([128, 2, PAD, PAD], f32r)
                pad_y = singles.tile([128, 2, PAD, PAD], f32r)
                wq_sb = singles.tile([128, 36 * 128], f32r)
                wk_sb = singles.tile([128, 36 * 128], f32r)
                wv_sb = singles.tile([128, 36 * 128], f32r)
                pq_sb = singles.tile([128, 2 * C], f32r)
                pk_sb = singles.tile([128, 2 * C], f32r)
                pv_sb = singles.tile([128, 2 * C], f32r)
                po_sb = singles.tile([128, 2 * C], f32r)

                ytr = yt.rearrange("(c p) m -> p c m", p=128)
                xtr = xt.rearrange("(c p) m -> p c m", p=128)
                wk4 = wk_sb[:].rearrange("p (a t f) -> p a t f", a=3, f=128)
                wq4 = wq_sb[:].rearrange("p (a t f) -> p a t f", a=3, f=128)
                wv4 = wv_sb[:].rearrange("p (a t f) -> p a t f", a=3, f=128)
                wckr = wck.rearrange("(a g) b c p f -> p a (g b c) f", a=3)
                wcqr = wcq.rearrange("(a g) b c p f -> p a (g b c) f", a=3)
                wcvr = wcv.rearrange("(a g) b c p f -> p a (g b c) f", a=3)
                # sync ring: pads for conv_k first; scalar ring: wk chunks —
                # both arrive in parallel so conv_k starts ~4.5us in.
                for ci in range(2):
                    nc.sync.dma_start(out=pad_y[:, ci], in_=ytr[:, ci])
                for a in range(3):
                    nc.scalar.dma_start(out=wk4[:, a], in_=wckr[:, a])
                for ci in range(2):
                    nc.scalar.dma_start(out=pad_x[:, ci], in_=xtr[:, ci])
                for a in range(3):
                    nc.sync.dma_start(out=wq4[:, a], in_=wcqr[:, a])
                for a in range(3):
                    nc.scalar.dma_start(out=wv4[:, a], in_=wcvr[:, a])
                for eng, psb, pdr in ((nc.sync, pk_sb, pk), (nc.sync, pv_sb, pv),
                                      (nc.scalar, pq_sb, pq), (nc.sync, po_sb, po)):
                    eng.dma_start(
                        out=psb[:].rearrange("p (t f) -> p t f", f=C),
                        in_=pdr.rearrange("t p f -> p t f"),
                    )

                # ---------- conv: raw = conv(img) in [c, L] layout ----------
                # raw tiles [128, 2048], col = chunk*1024 + l
                kraw = singles.tile([128, 2 * L], f32)
                vraw = singles.tile([128, 2 * L], f32)
                qraw = singles.tile([128, 2 * L], f32)
                st = statsp.tile([128, 12], f32)  # local (mean, m2) pairs

                def conv(pad_t, w_sb, raw, stat_base):
                    for co in range(2):
                        for half in range(2):
                            ps = psp.tile([128, 512], f32, tag="ps")
                            idx = 0
                            for kp in range(9):
                                ky, kx = kp // 3, kp % 3
                                for ci in range(2):
                                    blk = (kp * 2 + ci) * 2 + co
                                    lhsT = w_sb[:, blk * 128:(blk + 1) * 128]
                                    rhs = pad_t[:, ci, ky + half * 16: ky + half * 16 + 16,
                                                kx: kx + 32]
                                    nc.tensor.matmul(ps[:], lhsT, rhs,
                                                     start=(idx == 0), stop=(idx == 17))
                                    idx += 1
                            nc.vector.tensor_copy(
                                out=raw[:, co * L + half * 512: co * L + (half + 1) * 512].bitcast(f32r),
                                in_=ps[:])
                    # local BN statistics per chunk -> st cols (mean, m2)
                    for ch in range(2):
                        k = stat_base + ch
                        st6 = bnstp.tile([128, 2, 6], f32, tag="st6")
                        nc.vector.bn_stats(st6[:, 0, :], raw[:, ch * L: ch * L + 512])
                        nc.vector.bn_stats(st6[:, 1, :], raw[:, ch * L + 512: ch * L + 1024])
                        nc.vector.bn_aggr(st[:, 2 * k: 2 * k + 2], st6[:])
                        # m2 = mean^2 + var  (in place on the var column)
                        nc.vector.scalar_tensor_tensor(
                            out=st[:, 2 * k + 1: 2 * k + 2],
                            in0=st[:, 2 * k: 2 * k + 1],
                            scalar=st[:, 2 * k: 2 * k + 1],
                            in1=st[:, 2 * k + 1: 2 * k + 2],
                            op0=ALU.mult, op1=ALU.add,
                        )

                conv(pad_y, wk_sb, kraw, 2)
                conv(pad_x, wq_sb, qraw, 0)

                # ---------- AllReduce #1: q+k stats (overlaps conv_v) --------
                # q and k are all the exp stream needs; the v path (CC2 +
                # v-projection) hides under the attention exps since only the
                # attn@V matmuls consume it and the PE can catch up.
                cc_in1 = dramp.tile([128, 8], f32)
                cc_out1 = dramp.tile([128, 8], f32)
                nc.sync.dma_start(out=cc_in1[:], in_=st[:, 0:8])
                nc.gpsimd.collective_compute(
                    "AllReduce", ALU.add,
                    replica_groups=[list(range(8))],
                    ins=[cc_in1[:].opt()], outs=[cc_out1[:].opt()],
                )
                gstats = statsp.tile([128, 12], f32)
                nc.sync.dma_start(out=gstats[:, 0:8], in_=cc_out1[:])

                conv(pad_y, wv_sb, vraw, 4)

                # ---------- AllReduce #2: v stats ----------
                cc_in2 = dramp.tile([128, 4], f32)
                cc_out2 = dramp.tile([128, 4], f32)
                nc.sync.dma_start(out=cc_in2[:], in_=st[:, 8:12])
                nc.gpsimd.collective_compute(
                    "AllReduce", ALU.add,
                    replica_groups=[list(range(8))],
                    ins=[cc_in2[:].opt()], outs=[cc_out2[:].opt()],
                )
                nc.sync.dma_start(out=gstats[:, 8:12], in_=cc_out2[:])

                if DEBUG:
                    nc.sync.dma_start(out=dbg["dkraw"][:], in_=kraw[:])
                    nc.sync.dma_start(out=dbg["dst"][:], in_=st[:])

                # ---------- global scale/shift ----------
                var_t = statsp.tile([128, 6], f32)
                scale_t = statsp.tile([128, 6], f32)
                shift_t = statsp.tile([128, 6], f32)

                def bn_post(k0, nk):
                    seg = gstats[:, 2 * k0: 2 * (k0 + nk)]
                    nc.vector.tensor_scalar_mul(seg, seg, 1.0 / 8.0)
                    g2 = seg.rearrange("p (k two) -> p k two", two=2)
                    gmean = g2[:, :, 0]
                    gm2 = g2[:, :, 1]
                    vt = var_t[:, k0: k0 + nk]
                    nc.vector.tensor_mul(vt, gmean, gmean)
                    nc.vector.tensor_sub(vt, gm2, vt)
                    # rstd = exp(-0.5 * ln(var + eps)); ln+exp share one ACT
                    # table set so the big attention exps need no reload
                    nc.scalar.activation(vt, vt, AF.Ln, bias=epst[:, 0:1], scale=1.0)
                    nc.scalar.activation(vt, vt, AF.Exp, scale=-0.5)
                    sc = scale_t[:, k0: k0 + nk]
                    sh = shift_t[:, k0: k0 + nk]
                    nc.vector.tensor_mul(sc, vt, gbt[:, k0: k0 + nk])
                    nc.vector.tensor_mul(sh, gmean, sc)
                    nc.vector.tensor_sub(sh, gbt[:, 6 + k0: 6 + k0 + nk], sh)

                def bn_apply(raw, base):
                    for ch in range(2):
                        k = base + ch
                        nc.vector.tensor_scalar(
                            out=raw[:, ch * L:(ch + 1) * L].bitcast(f32r),
                            in0=raw[:, ch * L:(ch + 1) * L],
                            scalar1=scale_t[:, k: k + 1],
                            scalar2=shift_t[:, k: k + 1],
                            op0=ALU.mult, op1=ALU.add,
                        )

                bn_post(0, 4)   # q, k (CC1 results; overlaps conv_v / CC2)
                bn_apply(qraw, 0)
                bn_apply(kraw, 2)

                if DEBUG:
                    nc.sync.dma_start(out=dbg["dgst"][:], in_=gstats[:])
                    nc.sync.dma_start(out=dbg["dscale"][:], in_=scale_t[:])
                    nc.sync.dma_start(out=dbg["dshift"][:], in_=shift_t[:])
                    nc.sync.dma_start(out=dbg["dkbn"][:], in_=kraw[:])

                # ---------- q/k projections -> transposed [c, L] ----------
                qT = singles.tile([128, 2 * L], f32)
                kT = singles.tile([128, 2 * L], f32)

                def proj_T(src_t, wsb, dst, co):
                    for lh in range(2):
                        ps = psp.tile([128, 512], f32, tag="ps")
                        for ci in range(2):
                            lhsT = wsb[:, ci * C + co * 128: ci * C + (co + 1) * 128]
                            rhs = src_t[:, ci * L + lh * 512: ci * L + (lh + 1) * 512].bitcast(f32r)
                            nc.tensor.matmul(ps[:], lhsT, rhs,
                                             start=(ci == 0), stop=(ci == 1))
                        nc.scalar.copy(
                            dst[:, co * L + lh * 512: co * L + (lh + 1) * 512].bitcast(f32r),
                            ps[:])

                # co-chunk-interleaved so attention group g=0 (which needs the
                # co=0 halves of BOTH kT and qT) is fed first
                for co in range(2):
                    proj_T(kraw, pk_sb, kT, co)   # overlaps conv_v / CC2
                    proj_T(qraw, pq_sb, qT, co)
                bn_post(4, 2)               # v (CC2 results)
                bn_apply(vraw, 4)

                # ---------- v projection -> [t, c] layout, bf16 ----------
                v_sb = singles.tile([128, 8 * C], bf16)  # col = tc*256 + co
                for lt in range(8):
                    ps = psp.tile([128, C], f32, tag="ps")
                    for ci in range(2):
                        lhsT = vraw[:, ci * L + lt * 128: ci * L + (lt + 1) * 128].bitcast(f32r)
                        rhs = pv_sb[:, ci * C:(ci + 1) * C]
                        nc.tensor.matmul(ps[:], lhsT, rhs, start=(ci == 0), stop=(ci == 1))
                    nc.vector.tensor_copy(out=v_sb[:, lt * C:(lt + 1) * C], in_=ps[:])

                if DEBUG:
                    nc.sync.dma_start(out=dbg["dqT"][:], in_=qT[:])
                    nc.sync.dma_start(out=dbg["dkT"][:], in_=kT[:])
                    nc.sync.dma_start(out=dbg["dvsb"][:], in_=v_sb[:])

                if VARIANT == "convonly":
                    nc.sync.dma_start(
                        out=out.rearrange("(c p) l -> p c l", p=128),
                        in_=kraw[:].rearrange("p (c l) -> p c l", l=L))
                    continue
                if VARIANT == "noattn":
                    nc.sync.dma_start(
                        out=out.rearrange("(c p) l -> p c l", p=128),
                        in_=qT[:].rearrange("p (c l) -> p c l", l=L))
                    continue

                # ---------- attention ----------
                # Score tiles hold a PAIR of heads, one PSUM bank per head, so
                # every matmul owns its bank (start=True bank-clears are safe)
                # while 2-bank tiles leave room to triple-buffer: the PE runs
                # ahead of the exp instead of ping-ponging with it.
                attn_oT = singles.tile([128, 2 * L], f32)  # col = chunk*1024 + l
                for g in range(2):
                    for lh in range(2):
                        av = psp.tile([128, 512], f32, tag="ps")
                        den = psp.tile([128, 512], f32, tag="ps")
                        nc.vector.memset(av[:], 0.0)
                        nc.vector.memset(den[:], 0.0)
                        for tcp in range(4):  # pairs of t-chunks
                            # scores are staged (bf16) to SBUF by VectorE so
                            # the exp runs as one [128, 4096] op per t-chunk
                            # pair: 16 activation ops instead of 64 amortizes
                            # ScalarE's per-op overhead (73us -> 59us).
                            stg = stgp.tile([128, 2, 2, 1024], bf16, tag="stg")
                            for tci in range(2):
                                tc_i = 2 * tcp + tci
                                for jp in range(2):  # head pairs (0,1), (2,3)
                                    score = scorep.tile([128, 2 * 512], f32, tag="score")
                                    for jj in range(2):
                                        j = 2 * jp + jj
                                        lhsT = kT[32 * j: 32 * j + 32,
                                                  g * L + tc_i * 128: g * L + (tc_i + 1) * 128].bitcast(f32r)
                                        rhs = qT[32 * j: 32 * j + 32,
                                                 g * L + lh * 512: g * L + (lh + 1) * 512].bitcast(f32r)
                                        nc.tensor.matmul(score[:, jj * 512:(jj + 1) * 512],
                                                         lhsT, rhs, start=True, stop=True,
                                                         tile_position=(32 * j, 0))
                                    nc.vector.tensor_copy(out=stg[:, tci, jp, :], in_=score[:])
                            ptc = ptp.tile([128, 4096], bf16, tag="pt")
                            nc.scalar.activation(ptc[:], stg[:], AF.Exp, scale=ATT_SCALE)
                            # attn @ v and denominators for both t-chunks
                            for tci in range(2):
                                tc_i = 2 * tcp + tci
                                for j in range(4):
                                    rhs_pt = ptc[:, tci * 2048 + j * 512: tci * 2048 + (j + 1) * 512]
                                    lhsT_v = v_sb[:, tc_i * C + g * 128 + j * 32:
                                                  tc_i * C + g * 128 + (j + 1) * 32]
                                    nc.tensor.matmul(av[32 * j: 32 * j + 32, :], lhsT_v, rhs_pt,
                                                     start=False, stop=False,
                                                     tile_position=(0, 32 * j),
                                                     skip_group_check=True)
                                    nc.tensor.matmul(den[32 * j: 32 * j + 32, :], ones32[:], rhs_pt,
                                                     start=False, stop=False,
                                                     tile_position=(0, 32 * j),
                                                     skip_group_check=True)
                        rep = repp.tile([128, 512], f32, tag="rep")
                        nc.vector.reciprocal(rep[:], den[:])
                        nc.vector.tensor_mul(
                            attn_oT[:, g * L + lh * 512: g * L + (lh + 1) * 512].bitcast(f32r),
                            av[:], rep[:])

                if DEBUG:
                    nc.sync.dma_start(out=dbg["daoT"][:], in_=attn_oT[:])

                # ---------- output projection (transposed) + bias ----------
                out_sb = singles.tile([128, 2 * L], f32)
                for lh in range(2):
                    for co in range(2):
                        ps = psp.tile([128, 512], f32, tag="ps")
                        for ci in range(2):
                            lhsT = po_sb[:, ci * C + co * 128: ci * C + (co + 1) * 128]
                            rhs = attn_oT[:, ci * L + lh * 512: ci * L + (lh + 1) * 512].bitcast(f32r)
                            nc.tensor.matmul(ps[:], lhsT, rhs, start=(ci == 0), stop=(ci == 1))
                        nc.scalar.activation(
                            out_sb[:, co * L + lh * 512: co * L + (lh + 1) * 512],
                            ps[:], AF.Identity, bias=bot[:, co: co + 1], scale=1.0)

                outr = out.rearrange("(c p) l -> p c l", p=128)
                osr = out_sb[:].rearrange("p (c l) -> p c l", l=L)
                for lh in range(2):
                    nc.sync.dma_start(out=outr[:, :, lh * 512:(lh + 1) * 512],
                                      in_=osr[:, :, lh * 512:(lh + 1) * 512])

    nc.compile()
    return nc


def _prep_weights(conv_q_w, conv_k_w, conv_v_w, Wq, Wk, Wv, Wo,
                  bn_q_g, bn_q_b, bn_k_g, bn_k_b, bn_v_g, bn_v_b, bo):
    def conv_tiles(w):
        # [co, ci, ky, kx] -> [9, 2(ci), 2(co), 128, 128]
        t = np.ascontiguousarray(np.transpose(np.asarray(w, np.float32), (2, 3, 1, 0)))
        t = t.reshape(3, 3, 2, 128, 2, 128).transpose(0, 1, 2, 4, 3, 5)
        return np.ascontiguousarray(t.reshape(9, 2, 2, 128, 128))

    def proj_tiles(w):
        return np.ascontiguousarray(
            np.asarray(w, np.float32).T.reshape(2, 128, C))

    gbp = np.zeros((128, 12), np.float32)
    for i, (g, b) in enumerate(((bn_q_g, bn_q_b), (bn_k_g, bn_k_b), (bn_v_g, bn_v_b))):
        g = np.asarray(g, np.float32).reshape(2, 128)
        b = np.asarray(b, np.float32).reshape(2, 128)
        for ch in range(2):
            gbp[:, 2 * i + ch] = g[ch]
            gbp[:, 6 + 2 * i + ch] = b[ch]
    bop = np.ascontiguousarray(np.asarray(bo, np.float32).reshape(2, 128).T)
    return {
        "wcq": conv_tiles(conv_q_w), "wck": conv_tiles(conv_k_w),
        "wcv": conv_tiles(conv_v_w),
        "pq": proj_tiles(Wq), "pk": proj_tiles(Wk), "pv": proj_tiles(Wv),
        "po": proj_tiles(Wo),
        "gb": gbp, "bo": bop,
    }


def _get_nc(repeat=1):
    key = ("nc", repeat, VARIANT, DEBUG)
    if key not in _CACHE:
        _CACHE[key] = _build_nc(repeat)
    return _CACHE[key]


def run_spmd(in_maps, repeat=1, **kw):
    from concourse.bass_utils import run_bass_kernel_spmd
    return run_bass_kernel_spmd(_get_nc(repeat), in_maps, list(range(8)), **kw)


def _get_executor(repeat=1):
    """Build the sharded jitted callable once (mirrors
    bass2jax.run_bass_via_pjrt's multi-core path) so repeated calls skip
    retracing/compilation."""
    key = ("exec", repeat, VARIANT)
    if key in _CACHE:
        return _CACHE[key]
    import jax
    import numpy as _np
    from jax.sharding import Mesh, PartitionSpec
    from jax.experimental.shard_map import shard_map
    from concourse import bass2jax, mybir

    nc = _get_nc(repeat)
    bass2jax.install_neuronx_cc_hook()
    partition_name = nc.partition_id_tensor.name if nc.partition_id_tensor else None

    in_names, out_names, out_avals, zero_outs = [], [], [], []
    for alloc in nc.m.functions[0].allocations:
        if not isinstance(alloc, mybir.MemoryLocationSet):
            continue
        name = alloc.memorylocations[0].name
        if alloc.kind == "ExternalInput":
            if name != partition_name:
                in_names.append(name)
        elif alloc.kind == "ExternalOutput":
            dt_np = mybir.dt.np(alloc.dtype)
            shape = tuple(alloc.tensor_shape)
            out_avals.append(jax.core.ShapedArray(shape, dt_np))
            out_names.append(name)
            zero_outs.append(_np.zeros(shape, dt_np))

    n_params = len(in_names)
    n_outs = len(out_names)
    all_in_names = list(in_names) + list(out_names)
    if partition_name is not None:
        all_in_names.append(partition_name)
    donate = tuple(range(n_params, n_params + n_outs))

    def _body(*args):
        operands = list(args)
        if partition_name is not None:
            operands.append(bass2jax.partition_id_tensor())
        outs = bass2jax._bass_exec_p.bind(
            *operands,
            out_avals=tuple(out_avals),
            in_names=tuple(all_in_names),
            out_names=tuple(out_names),
            lowering_input_output_aliases=(),
            sim_require_finite=True,
            sim_require_nnan=True,
            nc=nc,
        )
        return tuple(outs)

    devices = jax.devices()[:B]
    mesh = Mesh(np.asarray(devices), ("core",))
    in_specs = (PartitionSpec("core"),) * (n_params + n_outs)
    out_specs = (PartitionSpec("core"),) * n_outs
    sharded = jax.jit(
        shard_map(_body, mesh=mesh, in_specs=in_specs, out_specs=out_specs,
                  check_rep=False),
        donate_argnums=donate, keep_unused=True,
    )
    _CACHE[("mesh", repeat, VARIANT)] = mesh
    _CACHE[("jit", repeat, VARIANT)] = sharded

    def run(in_maps):
        concat_in = [
            np.concatenate([np.asarray(in_maps[c][k]) for c in range(B)], axis=0)
            for k in in_names
        ]
        concat_zeros = [np.zeros((B * z.shape[0], *z.shape[1:]), z.dtype)
                        for z in zero_outs]
        out_arrs = sharded(*concat_in, *concat_zeros)
        return out_arrs, out_names, out_avals

    _CACHE[key] = run
    return run


def run_fast(in_maps, repeat=1):
    """Execute via the cached jitted callable; returns per-core dict list."""
    run = _get_executor(repeat)
    out_arrs, out_names, out_avals = run(in_maps)
    return [
        {name: np.asarray(out_arrs[i]).reshape(B, *out_avals[i].shape)[c]
         for i, name in enumerate(out_names)}
        for c in range(B)
    ]


def bench_wall(in_maps, repeat, n_iter):
    """Dispatch n_iter executions of the repeat-R NEFF with device-resident
    inputs and pre-staged donated zero buffers; return total wall seconds.
    Host/RPC overhead is identical across R, so (wall(R2)-wall(R1)) isolates
    device time."""
    import time as _time
    import jax
    from jax.sharding import NamedSharding, PartitionSpec

    _get_executor(repeat)  # ensure built
    nc = _get_nc(repeat)
    from concourse import mybir
    partition_name = nc.partition_id_tensor.name if nc.partition_id_tensor else None
    in_names, out_shapes = [], []
    for alloc in nc.m.functions[0].allocations:
        if not isinstance(alloc, mybir.MemoryLocationSet):
            continue
        name = alloc.memorylocations[0].name
        if alloc.kind == "ExternalInput" and name != partition_name:
            in_names.append(name)
        elif alloc.kind == "ExternalOutput":
            out_shapes.append((tuple(alloc.tensor_shape), mybir.dt.np(alloc.dtype)))

    key = ("bench_in", repeat, VARIANT)
    if key not in _CACHE:
        run = _CACHE[("exec", repeat, VARIANT)]
        # reach into the executor's jitted fn? rebuild inputs here instead
        mesh = _CACHE[("mesh", repeat, VARIANT)]
        sh = NamedSharding(mesh, PartitionSpec("core"))
        dev_in = [
            jax.device_put(
                np.concatenate([np.asarray(in_maps[c][k]) for c in range(B)], 0), sh)
            for k in in_names
        ]
        _CACHE[key] = (dev_in, sh)
    dev_in, sh = _CACHE[key]

    sharded = _CACHE[("jit", repeat, VARIANT)]
    # pre-stage donated zero sets
    zero_sets = []
    for _ in range(n_iter):
        zs = [jax.device_put(np.zeros((B * s[0], *s[1:]), dt), sh)
              for (s, dt) in out_shapes]
        zero_sets.append(zs)
    for zs in zero_sets:
        for z in zs:
            z.block_until_ready()

    outs = []
    t0 = _time.perf_counter()
    for it in range(n_iter):
        outs.append(sharded(*dev_in, *zero_sets[it]))
    for o in outs[-1]:
        o.block_until_ready()
    t1 = _time.perf_counter()
    return t1 - t0


def make_in_maps(x, y, h, w, conv_q_w, bn_q_g, bn_q_b,
                 conv_k_w, bn_k_g, bn_k_b, conv_v_w, bn_v_g, bn_v_b,
                 Wq, Wk, Wv, Wo, bo):
    assert int(h) == IMG and int(w) == IMG
    x = np.asarray(x, np.float32)
    y = np.asarray(y, np.float32)
    wmap = _prep_weights(conv_q_w, conv_k_w, conv_v_w, Wq, Wk, Wv, Wo,
                         bn_q_g, bn_q_b, bn_k_g, bn_k_b, bn_v_g, bn_v_b, bo)
    def pad_t(a):
        # [B, L, C] -> [B, C, 34*34] with zero border baked in
        at = np.transpose(a, (0, 2, 1)).reshape(B, C, IMG, IMG)
        ap = np.zeros((B, C, PAD, PAD), np.float32)
        ap[:, :, 1:33, 1:33] = at
        return ap.reshape(B, C, PAD * PAD)

    xT = pad_t(x)
    yT = pad_t(y)
    return [dict(wmap, xt=xT[b], yt=yT[b]) for b in range(B)]


def kernel(**inputs):
    in_maps = make_in_maps(**inputs)
    res = run_fast(in_maps)
    outs = [res[b]["out"] for b in range(B)]  # each [C, L]
    return np.ascontiguousarray(
        np.stack(outs, axis=0).transpose(0, 2, 1)).astype(np.float32)



# revision 6
# speedup vs baseline: 1.6327x; 1.6327x over previous
"""Distributed Trainium2 (Bass/Tile) kernel for nn_Attention_2D.

Pipeline (per batch element): 3x3 conv + BatchNorm (batch stats!) for
Q (from x), K, V (from y) -> linear projections -> multi-head attention
(scale = C**-0.5) -> output projection.

Sharding: data-parallel over batch B=8 across the 8 NeuronCores (one
image per core). The only cross-core dependency is the BatchNorm
mean/var over the whole batch -> tiny [128,{8,4}] AllReduces.

v2 schedule notes (the baseline staged scores through SBUF via VectorE
copies - 87us of DVE time - and ran conv_v before the projections,
pushing the 73us ScalarE exp stream to start ~50us in):
  - scores stay in PSUM ([128,1024] tiles, 2 banks each, 3 bufs) and
    ScalarE exps them PSUM->SBUF bf16 directly; no staging copies.
  - the exp stream is the attention bottleneck (64 x ~1.15us), so the
    emission order is built around starting it as early as possible and
    never starving it: conv_k, conv_q, CC1 (stats) covered by one
    quarter of conv_v, BN, q/k projections, then attention group
    (g=0,lh=0) begins; the remaining 3 quarters of conv_v + CC2 + the
    v projection are interleaved between score/exp units of that first
    block, whose attn@V matmuls are deferred (ptc tiles buffered) until
    v_sb lands. Later blocks run a lag-3 score->exp->attn@V pipeline.
  - BN rstd uses a DVE Newton iteration (bit-trick seed) instead of
    ScalarE Ln/Exp: the act-table loader thrashes sets otherwise (Ln
    and Exp resolve to different table sets -> 4 extra ~1.3us loads,
    two of them in front of the exp stream).
  - PSUM->SBUF copies (projections) and the output-proj bias add run on
    VectorE, keeping ScalarE exp-only.
  - optional: a fraction of exp tiles can run on VectorE via a
    Schraudolph bf16-bit-trick tensor_scalar (DVE_EXP_EVERY).
"""

import numpy as np

B, L, C = 8, 1024, 256
H = 8
D = 32  # head dim
IMG = 32  # h = w = 32
PAD = 34  # padded image side
EPS = 1e-5
ATT_SCALE = float(C) ** -0.5  # 1/16

# Schraudolph exp in bf16-bits: exp(s*x) ~= bitcast_bf16(i16(x*SCH_A + SCH_B))
SCH_A = (128.0 / float(np.log(2.0))) * ATT_SCALE
SCH_B = 128.0 * (127.0 - 0.0450466)

_CACHE = {}
DEBUG = False
VARIANT = "full"  # "full" | "noattn" | "convonly" (phase timing builds)
SIM_NO_CC = False  # replace AllReduce with local DMA copy (TimelineSim only)
RSTD_MODE = "newton"  # "newton" (DVE) | "lnexp" (ScalarE tables)
DVE_EXP_EVERY = 0  # 0 = off; n>0: every nth exp tile on DVE (Schraudolph)


def _build_nc(repeat=1):
    import concourse.bacc as bacc
    import concourse.tile as tile
    from concourse import mybir

    f32 = mybir.dt.float32
    f32r = mybir.dt.float32r
    bf16 = mybir.dt.bfloat16
    i16 = mybir.dt.int16
    i32 = mybir.dt.int32
    AF = mybir.ActivationFunctionType
    ALU = mybir.AluOpType

    nc = bacc.Bacc(None, target_bir_lowering=False)
    nc.num_devices = 8

    # ---- DRAM parameters (host-prepped layouts) ----
    xt = nc.declare_dram_parameter("xt", [C, PAD * PAD], f32r, isOutput=False)
    yt = nc.declare_dram_parameter("yt", [C, PAD * PAD], f32r, isOutput=False)
    # conv weights: [9(kpos), 2(ci), 2(co), 128, 128]
    wcq = nc.declare_dram_parameter("wcq", [9, 2, 2, 128, 128], f32r, isOutput=False)
    wck = nc.declare_dram_parameter("wck", [9, 2, 2, 128, 128], f32r, isOutput=False)
    wcv = nc.declare_dram_parameter("wcv", [9, 2, 2, 128, 128], f32r, isOutput=False)
    # projection weights W.T tiled: [2(ci), 128, 256(co)]
    pq = nc.declare_dram_parameter("pq", [2, 128, C], f32r, isOutput=False)
    pk = nc.declare_dram_parameter("pk", [2, 128, C], f32r, isOutput=False)
    pv = nc.declare_dram_parameter("pv", [2, 128, C], f32r, isOutput=False)
    po = nc.declare_dram_parameter("po", [2, 128, C], f32r, isOutput=False)
    # gamma/beta pack [128, 12]: cols 0-5 gamma, 6-11 beta
    gb = nc.declare_dram_parameter("gb", [128, 12], f32, isOutput=False)
    bo = nc.declare_dram_parameter("bo", [128, 2], f32, isOutput=False)
    out = nc.declare_dram_parameter("out", [C, L], f32, isOutput=True)

    with tile.TileContext(nc) as tc:
        with tc.tile_pool(name="singles", bufs=1) as singles, \
             tc.tile_pool(name="stats", bufs=1) as statsp, \
             tc.tile_pool(name="bnst", bufs=4) as bnstp, \
             tc.tile_pool(name="rep", bufs=2) as repp, \
             tc.tile_pool(name="pt", bufs=20) as ptp, \
             tc.tile_pool(name="ps", bufs=2, space="PSUM") as psp, \
             tc.tile_pool(name="score_ps", bufs=3, space="PSUM") as scorep, \
             tc.tile_pool(name="dram", bufs=1, space="DRAM") as dramp:

            for _rep in range(repeat):
                # ---------- constants / small tiles ----------
                ones32 = singles.tile([128, 32], bf16)
                nc.vector.memset(ones32[:], 1.0)
                epst = singles.tile([128, 1], f32)
                nc.vector.memset(epst[:], EPS)
                magict = singles.tile([128, 6], f32)
                # f32 whose bits are 0x5f375a86 (Newton-rsqrt magic)
                nc.vector.memset(magict[:], 1.3212019791402893e19)
                gbt = singles.tile([128, 12], f32)
                nc.sync.dma_start(out=gbt[:], in_=gb[:])
                bot = singles.tile([128, 2], f32)
                nc.sync.dma_start(out=bot[:], in_=bo[:])

                # ---------- padded images + weights ----------
                pad_x = singles.tile([128, 2, PAD, PAD], f32r)
                pad_y = singles.tile([128, 2, PAD, PAD], f32r)
                wq_sb = singles.tile([128, 36 * 128], f32r)
                wk_sb = singles.tile([128, 36 * 128], f32r)
                wv_sb = singles.tile([128, 36 * 128], f32r)
                pq_sb = singles.tile([128, 2 * C], f32r)
                pk_sb = singles.tile([128, 2 * C], f32r)
                pv_sb = singles.tile([128, 2 * C], f32r)
                po_sb = singles.tile([128, 2 * C], f32r)

                ytr = yt.rearrange("(c p) m -> p c m", p=128)
                xtr = xt.rearrange("(c p) m -> p c m", p=128)
                wk4 = wk_sb[:].rearrange("p (a t f) -> p a t f", a=3, f=128)
                wq4 = wq_sb[:].rearrange("p (a t f) -> p a t f", a=3, f=128)
                wv4 = wv_sb[:].rearrange("p (a t f) -> p a t f", a=3, f=128)
                wckr = wck.rearrange("(a g) b c p f -> p a (g b c) f", a=3)
                wcqr = wcq.rearrange("(a g) b c p f -> p a (g b c) f", a=3)
                wcvr = wcv.rearrange("(a g) b c p f -> p a (g b c) f", a=3)
                # two HWDGE rings, FIFO each; emit in consumption order:
                # sync: pad_y -> wcq -> pk -> pv ; scalar: wck -> pad_x -> pq
                # -> wcv -> po.  conv_k needs pad_y(sync)+wck(scalar) ~4.5us.
                for ci in range(2):
                    nc.sync.dma_start(out=pad_y[:, ci], in_=ytr[:, ci])
                for a in range(3):
                    nc.scalar.dma_start(out=wk4[:, a], in_=wckr[:, a])
                for a in range(3):
                    nc.sync.dma_start(out=wq4[:, a], in_=wcqr[:, a])
                for ci in range(2):
                    nc.scalar.dma_start(out=pad_x[:, ci], in_=xtr[:, ci])
                for eng, psb, pdr in ((nc.sync, pk_sb, pk), (nc.scalar, pq_sb, pq),
                                      (nc.sync, pv_sb, pv)):
                    eng.dma_start(
                        out=psb[:].rearrange("p (t f) -> p t f", f=C),
                        in_=pdr.rearrange("t p f -> p t f"),
                    )
                for a in range(3):
                    nc.scalar.dma_start(out=wv4[:, a], in_=wcvr[:, a])
                nc.scalar.dma_start(
                    out=po_sb[:].rearrange("p (t f) -> p t f", f=C),
                    in_=po.rearrange("t p f -> p t f"),
                )

                # ---------- conv machinery ----------
                kraw = singles.tile([128, 2 * L], f32)
                vraw = singles.tile([128, 2 * L], f32)
                qraw = singles.tile([128, 2 * L], f32)
                st = statsp.tile([128, 12], f32)  # local (mean, m2) pairs

                def conv_quarter(pad_t, w_sb, raw, stat_base, co, half):
                    ps = psp.tile([128, 512], f32, tag="ps")
                    idx = 0
                    for kp in range(9):
                        ky, kx = kp // 3, kp % 3
                        for ci in range(2):
                            blk = (kp * 2 + ci) * 2 + co
                            lhsT = w_sb[:, blk * 128:(blk + 1) * 128]
                            rhs = pad_t[:, ci, ky + half * 16: ky + half * 16 + 16,
                                        kx: kx + 32]
                            nc.tensor.matmul(ps[:], lhsT, rhs,
                                             start=(idx == 0), stop=(idx == 17))
                            idx += 1
                    nc.vector.tensor_copy(
                        out=raw[:, co * L + half * 512: co * L + (half + 1) * 512].bitcast(f32r),
                        in_=ps[:])
                    if half == 1:
                        k = stat_base + co
                        st6 = bnstp.tile([128, 2, 6], f32, tag="st6")
                        nc.vector.bn_stats(st6[:, 0, :], raw[:, co * L: co * L + 512])
                        nc.vector.bn_stats(st6[:, 1, :], raw[:, co * L + 512: co * L + 1024])
                        nc.vector.bn_aggr(st[:, 2 * k: 2 * k + 2], st6[:])
                        # m2 = mean^2 + var (in place on the var column)
                        nc.vector.scalar_tensor_tensor(
                            out=st[:, 2 * k + 1: 2 * k + 2],
                            in0=st[:, 2 * k: 2 * k + 1],
                            scalar=st[:, 2 * k: 2 * k + 1],
                            in1=st[:, 2 * k + 1: 2 * k + 2],
                            op0=ALU.mult, op1=ALU.add,
                        )

                def conv_full(pad_t, w_sb, raw, stat_base):
                    for co in range(2):
                        for half in range(2):
                            conv_quarter(pad_t, w_sb, raw, stat_base, co, half)

                conv_full(pad_y, wk_sb, kraw, 2)
                conv_full(pad_x, wq_sb, qraw, 0)

                # ---------- AllReduce #1: q+k stats ----------
                cc_in1 = dramp.tile([128, 8], f32)
                cc_out1 = dramp.tile([128, 8], f32)
                nc.sync.dma_start(out=cc_in1[:], in_=st[:, 0:8])
                if SIM_NO_CC:
                    nc.gpsimd.dma_start(out=cc_out1[:], in_=cc_in1[:])
                else:
                    nc.gpsimd.collective_compute(
                        "AllReduce", ALU.add,
                        replica_groups=[list(range(8))],
                        ins=[cc_in1[:].opt()], outs=[cc_out1[:].opt()],
                    )
                gstats = statsp.tile([128, 12], f32)
                nc.sync.dma_start(out=gstats[:, 0:8], in_=cc_out1[:])

                # first quarter of conv_v covers the CC1 latency
                if VARIANT == "full":
                    conv_quarter(pad_y, wv_sb, vraw, 4, 0, 0)
                else:
                    conv_full(pad_y, wv_sb, vraw, 4)

                # ---------- global scale/shift ----------
                var_t = statsp.tile([128, 6], f32)
                nwt_h = statsp.tile([128, 6], f32)
                nwt_y = statsp.tile([128, 6], f32)
                nwt_t = statsp.tile([128, 6], f32)
                scale_t = statsp.tile([128, 6], f32)
                shift_t = statsp.tile([128, 6], f32)

                def bn_post(k0, nk):
                    seg = gstats[:, 2 * k0: 2 * (k0 + nk)]
                    nc.vector.tensor_scalar_mul(seg, seg, 1.0 / 8.0)
                    g2 = seg.rearrange("p (k two) -> p k two", two=2)
                    gmean = g2[:, :, 0]
                    gm2 = g2[:, :, 1]
                    vt = var_t[:, k0: k0 + nk]
                    nc.vector.tensor_mul(vt, gmean, gmean)
                    nc.vector.tensor_sub(vt, gm2, vt)
                    if RSTD_MODE == "newton":
                        # rstd = rsqrt(var+eps): bit-trick seed + 2 Newton steps
                        nc.vector.tensor_scalar_add(vt, vt, EPS)
                        hv = nwt_h[:, k0: k0 + nk]
                        nc.vector.tensor_scalar_mul(hv, vt, 0.5)
                        yv = nwt_y[:, k0: k0 + nk]
                        nc.vector.tensor_scalar(
                            out=yv.bitcast(i32), in0=vt.bitcast(i32),
                            scalar1=1, scalar2=None, op0=ALU.logical_shift_right)
                        nc.vector.tensor_sub(
                            yv.bitcast(i32), magict[:, k0: k0 + nk].bitcast(i32),
                            yv.bitcast(i32))
                        tv = nwt_t[:, k0: k0 + nk]
                        for _ in range(2):
                            nc.vector.tensor_mul(tv, yv, yv)
                            nc.vector.tensor_mul(tv, tv, hv)
                            nc.vector.tensor_scalar(
                                out=tv, in0=tv, scalar1=-1.0, scalar2=1.5,
                                op0=ALU.mult, op1=ALU.add)
                            nc.vector.tensor_mul(yv, yv, tv)
                        vt = yv
                    else:
                        nc.scalar.activation(vt, vt, AF.Ln, bias=epst[:, 0:1], scale=1.0)
                        nc.scalar.activation(vt, vt, AF.Exp, scale=-0.5)
                    sc = scale_t[:, k0: k0 + nk]
                    sh = shift_t[:, k0: k0 + nk]
                    nc.vector.tensor_mul(sc, vt, gbt[:, k0: k0 + nk])
                    nc.vector.tensor_mul(sh, gmean, sc)
                    nc.vector.tensor_sub(sh, gbt[:, 6 + k0: 6 + k0 + nk], sh)

                def bn_apply(raw, base):
                    for ch in range(2):
                        k = base + ch
                        nc.vector.tensor_scalar(
                            out=raw[:, ch * L:(ch + 1) * L].bitcast(f32r),
                            in0=raw[:, ch * L:(ch + 1) * L],
                            scalar1=scale_t[:, k: k + 1],
                            scalar2=shift_t[:, k: k + 1],
                            op0=ALU.mult, op1=ALU.add,
                        )

                bn_post(0, 4)   # q, k
                bn_apply(qraw, 0)
                bn_apply(kraw, 2)

                if VARIANT == "convonly":
                    cc_in2 = dramp.tile([128, 4], f32)
                    cc_out2 = dramp.tile([128, 4], f32)
                    nc.sync.dma_start(out=cc_in2[:], in_=st[:, 8:12])
                    if SIM_NO_CC:
                        nc.gpsimd.dma_start(out=cc_out2[:], in_=cc_in2[:])
                    else:
                        nc.gpsimd.collective_compute(
                            "AllReduce", ALU.add,
                            replica_groups=[list(range(8))],
                            ins=[cc_in2[:].opt()], outs=[cc_out2[:].opt()],
                        )
                    nc.sync.dma_start(out=gstats[:, 8:12], in_=cc_out2[:])
                    bn_post(4, 2)
                    bn_apply(vraw, 4)
                    nc.sync.dma_start(
                        out=out.rearrange("(c p) l -> p c l", p=128),
                        in_=kraw[:].rearrange("p (c l) -> p c l", l=L))
                    continue

                # ---------- q/k projections -> transposed [c, L] ----------
                qT = singles.tile([128, 2 * L], f32)
                kT = singles.tile([128, 2 * L], f32)

                def proj_T(src_t, wsb, dst, co):
                    for lh in range(2):
                        ps = psp.tile([128, 512], f32, tag="ps")
                        for ci in range(2):
                            lhsT = wsb[:, ci * C + co * 128: ci * C + (co + 1) * 128]
                            rhs = src_t[:, ci * L + lh * 512: ci * L + (lh + 1) * 512].bitcast(f32r)
                            nc.tensor.matmul(ps[:], lhsT, rhs,
                                             start=(ci == 0), stop=(ci == 1))
                        nc.vector.tensor_copy(
                            out=dst[:, co * L + lh * 512: co * L + (lh + 1) * 512].bitcast(f32r),
                            in_=ps[:])

                for co in range(2):
                    proj_T(kraw, pk_sb, kT, co)
                    proj_T(qraw, pq_sb, qT, co)

                # ---------- v path helpers (emitted later, interleaved) ----
                v_sb = singles.tile([128, 8 * C], bf16)  # col = tc*256 + co

                def emit_cc2_bn_v():
                    cc_in2 = dramp.tile([128, 4], f32)
                    cc_out2 = dramp.tile([128, 4], f32)
                    nc.sync.dma_start(out=cc_in2[:], in_=st[:, 8:12])
                    if SIM_NO_CC:
                        nc.gpsimd.dma_start(out=cc_out2[:], in_=cc_in2[:])
                    else:
                        nc.gpsimd.collective_compute(
                            "AllReduce", ALU.add,
                            replica_groups=[list(range(8))],
                            ins=[cc_in2[:].opt()], outs=[cc_out2[:].opt()],
                        )
                    nc.sync.dma_start(out=gstats[:, 8:12], in_=cc_out2[:])
                    bn_post(4, 2)
                    bn_apply(vraw, 4)

                def emit_vproj():
                    for lt in range(8):
                        ps = psp.tile([128, C], f32, tag="ps")
                        for ci in range(2):
                            lhsT = vraw[:, ci * L + lt * 128: ci * L + (lt + 1) * 128].bitcast(f32r)
                            rhs = pv_sb[:, ci * C:(ci + 1) * C]
                            nc.tensor.matmul(ps[:], lhsT, rhs, start=(ci == 0), stop=(ci == 1))
                        nc.vector.tensor_copy(out=v_sb[:, lt * C:(lt + 1) * C], in_=ps[:])

                if VARIANT == "noattn":
                    emit_cc2_bn_v()
                    emit_vproj()
                    nc.sync.dma_start(
                        out=out.rearrange("(c p) l -> p c l", p=128),
                        in_=qT[:].rearrange("p (c l) -> p c l", l=L))
                    continue

                # ---------- attention ----------
                attn_oT = singles.tile([128, 2 * L], f32)  # col = g*1024 + l
                exp_ctr = [0]

                def sc_unit(g, lh, tc_i, jp):
                    score = scorep.tile([128, 1024], f32, tag="score")
                    for jj in range(2):
                        j = 2 * jp + jj
                        lhsT = kT[32 * j: 32 * j + 32,
                                  g * L + tc_i * 128: g * L + (tc_i + 1) * 128].bitcast(f32r)
                        rhs = qT[32 * j: 32 * j + 32,
                                 g * L + lh * 512: g * L + (lh + 1) * 512].bitcast(f32r)
                        nc.tensor.matmul(score[:, jj * 512:(jj + 1) * 512],
                                         lhsT, rhs, start=True, stop=True,
                                         tile_position=(32 * j, 0))
                    return score

                def exp_unit(score):
                    ptc = ptp.tile([128, 1024], bf16, tag="pt")
                    exp_ctr[0] += 1
                    if DVE_EXP_EVERY and exp_ctr[0] % DVE_EXP_EVERY == 0:
                        nc.vector.tensor_scalar(
                            out=ptc[:].bitcast(i16), in0=score[:],
                            scalar1=SCH_A, scalar2=SCH_B,
                            op0=ALU.mult, op1=ALU.add)
                    else:
                        nc.scalar.activation(ptc[:], score[:], AF.Exp, scale=ATT_SCALE)
                    return ptc

                def av_unit(av, den, ptc, g, tc_i, jp):
                    for jj in range(2):
                        j = 2 * jp + jj
                        rhs_pt = ptc[:, jj * 512:(jj + 1) * 512]
                        lhsT_v = v_sb[:, tc_i * C + g * 128 + j * 32:
                                      tc_i * C + g * 128 + (j + 1) * 32]
                        nc.tensor.matmul(av[32 * j: 32 * j + 32, :], lhsT_v, rhs_pt,
                                         start=False, stop=False,
                                         tile_position=(0, 32 * j),
                                         skip_group_check=True)
                    for jj in range(2):
                        j = 2 * jp + jj
                        rhs_pt = ptc[:, jj * 512:(jj + 1) * 512]
                        nc.tensor.matmul(den[32 * j: 32 * j + 32, :], ones32[:], rhs_pt,
                                         start=False, stop=False,
                                         tile_position=(0, 32 * j),
                                         skip_group_check=True)

                def new_avden():
                    av = psp.tile([128, 512], f32, tag="ps")
                    den = psp.tile([128, 512], f32, tag="ps")
                    nc.vector.memset(av[:], 0.0)
                    nc.vector.memset(den[:], 0.0)
                    return av, den

                def norm_block(av, den, g, lh):
                    rep = repp.tile([128, 512], f32, tag="rep")
                    nc.vector.reciprocal_approx_fast(out=rep[:], in_=den[:])
                    nc.vector.tensor_mul(
                        attn_oT[:, g * L + lh * 512: g * L + (lh + 1) * 512].bitcast(f32r),
                        av[:], rep[:])

                units = [(t, jp) for t in range(8) for jp in range(2)]

                # --- block (g=0, lh=0): exps buffered, attn@V deferred ---
                # conv_v quarters (co,half) = (0,1),(1,0),(1,1) slot between
                # units so the PE keeps pace with the ScalarE exp stream.
                fillers = {
                    4: lambda: conv_quarter(pad_y, wv_sb, vraw, 4, 0, 1),
                    9: lambda: conv_quarter(pad_y, wv_sb, vraw, 4, 1, 0),
                    13: lambda: conv_quarter(pad_y, wv_sb, vraw, 4, 1, 1),
                }
                backlog = []
                for u, (t, jp) in enumerate(units):
                    score = sc_unit(0, 0, t, jp)
                    backlog.append((exp_unit(score), t, jp))
                    if u in fillers:
                        fillers[u]()
                emit_cc2_bn_v()
                # keep the exp stream fed while v catches up
                lead = [(exp_unit(sc_unit(0, 1, t, jp)), t, jp)
                        for (t, jp) in units[:2]]
                emit_vproj()
                av0, den0 = new_avden()
                for ptc, t, jp in backlog:
                    av_unit(av0, den0, ptc, 0, t, jp)
                norm_block(av0, den0, 0, 0)

                # --- remaining blocks: lag-3 pipeline ---
                def run_block(g, lh, pending):
                    av, den = new_avden()
                    for (t, jp) in units[len(pending):]:
                        score = sc_unit(g, lh, t, jp)
                        pending.append((exp_unit(score), t, jp))
                        if len(pending) > 3:
                            ptc, pt_, pjp = pending.pop(0)
                            av_unit(av, den, ptc, g, pt_, pjp)
                    while pending:
                        ptc, pt_, pjp = pending.pop(0)
                        av_unit(av, den, ptc, g, pt_, pjp)
                    norm_block(av, den, g, lh)

                run_block(0, 1, lead)
                run_block(1, 0, [])
                run_block(1, 1, [])

                # ---------- output projection (transposed) + bias ----------
                out_sb = singles.tile([128, 2 * L], f32)
                for lh in range(2):
                    for co in range(2):
                        ps = psp.tile([128, 512], f32, tag="ps")
                        for ci in range(2):
                            lhsT = po_sb[:, ci * C + co * 128: ci * C + (co + 1) * 128]
                            rhs = attn_oT[:, ci * L + lh * 512: ci * L + (lh + 1) * 512].bitcast(f32r)
                            nc.tensor.matmul(ps[:], lhsT, rhs, start=(ci == 0), stop=(ci == 1))
                        nc.vector.tensor_scalar(
                            out=out_sb[:, co * L + lh * 512: co * L + (lh + 1) * 512],
                            in0=ps[:], scalar1=bot[:, co: co + 1], scalar2=None,
                            op0=ALU.add)

                outr = out.rearrange("(c p) l -> p c l", p=128)
                osr = out_sb[:].rearrange("p (c l) -> p c l", l=L)
                for lh in range(2):
                    nc.sync.dma_start(out=outr[:, :, lh * 512:(lh + 1) * 512],
                                      in_=osr[:, :, lh * 512:(lh + 1) * 512])

    nc.compile()
    return nc


def _prep_weights(conv_q_w, conv_k_w, conv_v_w, Wq, Wk, Wv, Wo,
                  bn_q_g, bn_q_b, bn_k_g, bn_k_b, bn_v_g, bn_v_b, bo):
    def conv_tiles(w):
        # [co, ci, ky, kx] -> [9, 2(ci), 2(co), 128, 128]
        t = np.ascontiguousarray(np.transpose(np.asarray(w, np.float32), (2, 3, 1, 0)))
        t = t.reshape(3, 3, 2, 128, 2, 128).transpose(0, 1, 2, 4, 3, 5)
        return np.ascontiguousarray(t.reshape(9, 2, 2, 128, 128))

    def proj_tiles(w):
        return np.ascontiguousarray(
            np.asarray(w, np.float32).T.reshape(2, 128, C))

    gbp = np.zeros((128, 12), np.float32)
    for i, (g, b) in enumerate(((bn_q_g, bn_q_b), (bn_k_g, bn_k_b), (bn_v_g, bn_v_b))):
        g = np.asarray(g, np.float32).reshape(2, 128)
        b = np.asarray(b, np.float32).reshape(2, 128)
        for ch in range(2):
            gbp[:, 2 * i + ch] = g[ch]
            gbp[:, 6 + 2 * i + ch] = b[ch]
    bop = np.ascontiguousarray(np.asarray(bo, np.float32).reshape(2, 128).T)
    return {
        "wcq": conv_tiles(conv_q_w), "wck": conv_tiles(conv_k_w),
        "wcv": conv_tiles(conv_v_w),
        "pq": proj_tiles(Wq), "pk": proj_tiles(Wk), "pv": proj_tiles(Wv),
        "po": proj_tiles(Wo),
        "gb": gbp, "bo": bop,
    }


def _get_nc(repeat=1):
    key = ("nc", repeat, VARIANT, DEBUG, RSTD_MODE, DVE_EXP_EVERY)
    if key not in _CACHE:
        _CACHE[key] = _build_nc(repeat)
    return _CACHE[key]


def run_spmd(in_maps, repeat=1, **kw):
    from concourse.bass_utils import run_bass_kernel_spmd
    return run_bass_kernel_spmd(_get_nc(repeat), in_maps, list(range(8)), **kw)


def _get_executor(repeat=1):
    """Build the sharded jitted callable once (mirrors
    bass2jax.run_bass_via_pjrt's multi-core path) so repeated calls skip
    retracing/compilation."""
    key = ("exec", repeat, VARIANT, RSTD_MODE, DVE_EXP_EVERY)
    if key in _CACHE:
        return _CACHE[key]
    import jax
    import numpy as _np
    from jax.sharding import Mesh, PartitionSpec
    from jax.experimental.shard_map import shard_map
    from concourse import bass2jax, mybir

    nc = _get_nc(repeat)
    bass2jax.install_neuronx_cc_hook()
    partition_name = nc.partition_id_tensor.name if nc.partition_id_tensor else None

    in_names, out_names, out_avals, zero_outs = [], [], [], []
    for alloc in nc.m.functions[0].allocations:
        if not isinstance(alloc, mybir.MemoryLocationSet):
            continue
        name = alloc.memorylocations[0].name
        if alloc.kind == "ExternalInput":
            if name != partition_name:
                in_names.append(name)
        elif alloc.kind == "ExternalOutput":
            dt_np = mybir.dt.np(alloc.dtype)
            shape = tuple(alloc.tensor_shape)
            out_avals.append(jax.core.ShapedArray(shape, dt_np))
            out_names.append(name)
            zero_outs.append(_np.zeros(shape, dt_np))

    n_params = len(in_names)
    n_outs = len(out_names)
    all_in_names = list(in_names) + list(out_names)
    if partition_name is not None:
        all_in_names.append(partition_name)
    donate = tuple(range(n_params, n_params + n_outs))

    def _body(*args):
        operands = list(args)
        if partition_name is not None:
            operands.append(bass2jax.partition_id_tensor())
        outs = bass2jax._bass_exec_p.bind(
            *operands,
            out_avals=tuple(out_avals),
            in_names=tuple(all_in_names),
            out_names=tuple(out_names),
            lowering_input_output_aliases=(),
            sim_require_finite=True,
            sim_require_nnan=True,
            nc=nc,
        )
        return tuple(outs)

    devices = jax.devices()[:B]
    mesh = Mesh(np.asarray(devices), ("core",))
    in_specs = (PartitionSpec("core"),) * (n_params + n_outs)
    out_specs = (PartitionSpec("core"),) * n_outs
    sharded = jax.jit(
        shard_map(_body, mesh=mesh, in_specs=in_specs, out_specs=out_specs,
                  check_rep=False),
        donate_argnums=donate, keep_unused=True,
    )
    _CACHE[("mesh", repeat, VARIANT)] = mesh
    _CACHE[("jit", repeat, VARIANT)] = sharded

    def run(in_maps):
        concat_in = [
            np.concatenate([np.asarray(in_maps[c][k]) for c in range(B)], axis=0)
            for k in in_names
        ]
        concat_zeros = [np.zeros((B * z.shape[0], *z.shape[1:]), z.dtype)
                        for z in zero_outs]
        out_arrs = sharded(*concat_in, *concat_zeros)
        return out_arrs, out_names, out_avals

    _CACHE[key] = run
    return run


def run_fast(in_maps, repeat=1):
    """Execute via the cached jitted callable; returns per-core dict list."""
    run = _get_executor(repeat)
    out_arrs, out_names, out_avals = run(in_maps)
    return [
        {name: np.asarray(out_arrs[i]).reshape(B, *out_avals[i].shape)[c]
         for i, name in enumerate(out_names)}
        for c in range(B)
    ]


def bench_wall(in_maps, repeat, n_iter):
    """Dispatch n_iter executions of the repeat-R NEFF with device-resident
    inputs and pre-staged donated zero buffers; return total wall seconds."""
    import time as _time
    import jax
    from jax.sharding import NamedSharding, PartitionSpec

    _get_executor(repeat)  # ensure built
    nc = _get_nc(repeat)
    from concourse import mybir
    partition_name = nc.partition_id_tensor.name if nc.partition_id_tensor else None
    in_names, out_shapes = [], []
    for alloc in nc.m.functions[0].allocations:
        if not isinstance(alloc, mybir.MemoryLocationSet):
            continue
        name = alloc.memorylocations[0].name
        if alloc.kind == "ExternalInput" and name != partition_name:
            in_names.append(name)
        elif alloc.kind == "ExternalOutput":
            out_shapes.append((tuple(alloc.tensor_shape), mybir.dt.np(alloc.dtype)))

    key = ("bench_in", repeat, VARIANT)
    if key not in _CACHE:
        mesh = _CACHE[("mesh", repeat, VARIANT)]
        sh = NamedSharding(mesh, PartitionSpec("core"))
        dev_in = [
            jax.device_put(
                np.concatenate([np.asarray(in_maps[c][k]) for c in range(B)], 0), sh)
            for k in in_names
        ]
        _CACHE[key] = (dev_in, sh)
    dev_in, sh = _CACHE[key]

    sharded = _CACHE[("jit", repeat, VARIANT)]
    zero_sets = []
    for _ in range(n_iter):
        zs = [jax.device_put(np.zeros((B * s[0], *s[1:]), dt), sh)
              for (s, dt) in out_shapes]
        zero_sets.append(zs)
    for zs in zero_sets:
        for z in zs:
            z.block_until_ready()

    outs = []
    t0 = _time.perf_counter()
    for it in range(n_iter):
        outs.append(sharded(*dev_in, *zero_sets[it]))
    for o in outs[-1]:
        o.block_until_ready()
    t1 = _time.perf_counter()
    return t1 - t0


def make_in_maps(x, y, h, w, conv_q_w, bn_q_g, bn_q_b,
                 conv_k_w, bn_k_g, bn_k_b, conv_v_w, bn_v_g, bn_v_b,
                 Wq, Wk, Wv, Wo, bo):
    assert int(h) == IMG and int(w) == IMG
    x = np.asarray(x, np.float32)
    y = np.asarray(y, np.float32)
    wmap = _prep_weights(conv_q_w, conv_k_w, conv_v_w, Wq, Wk, Wv, Wo,
                         bn_q_g, bn_q_b, bn_k_g, bn_k_b, bn_v_g, bn_v_b, bo)
    def pad_t(a):
        # [B, L, C] -> [B, C, 34*34] with zero border baked in
        at = np.transpose(a, (0, 2, 1)).reshape(B, C, IMG, IMG)
        ap = np.zeros((B, C, PAD, PAD), np.float32)
        ap[:, :, 1:33, 1:33] = at
        return ap.reshape(B, C, PAD * PAD)

    xT = pad_t(x)
    yT = pad_t(y)
    return [dict(wmap, xt=xT[b], yt=yT[b]) for b in range(B)]


def kernel(**inputs):
    in_maps = make_in_maps(**inputs)
    res = run_fast(in_maps)
    outs = [res[b]["out"] for b in range(B)]  # each [C, L]
    return np.ascontiguousarray(
        np.stack(outs, axis=0).transpose(0, 2, 1)).astype(np.float32)


# revision 8
# speedup vs baseline: 3.2606x; 1.9970x over previous
"""Distributed Trainium2 (Bass/Tile) kernel for nn_Attention_2D.

Pipeline (per batch element): 3x3 conv + BatchNorm (batch stats!) for
Q (from x), K, V (from y) -> linear projections -> multi-head attention
(scale = C**-0.5) -> output projection.

Sharding: data-parallel over batch B=8 across the 8 NeuronCores (one
image per core). The only cross-core dependency is the BatchNorm
mean/var over the whole batch -> tiny [128,{8,4}] AllReduces.

v2 schedule notes (the baseline staged scores through SBUF via VectorE
copies - 87us of DVE time - and ran conv_v before the projections,
pushing the 73us ScalarE exp stream to start ~50us in):
  - scores stay in PSUM ([128,1024] tiles, 2 banks each, 3 bufs) and
    ScalarE exps them PSUM->SBUF bf16 directly; no staging copies.
  - the exp stream is the attention bottleneck (64 x ~1.15us), so the
    emission order is built around starting it as early as possible and
    never starving it: conv_k, conv_q, CC1 (stats) covered by one
    quarter of conv_v, BN, q/k projections, then attention group
    (g=0,lh=0) begins; the remaining 3 quarters of conv_v + CC2 + the
    v projection are interleaved between score/exp units of that first
    block, whose attn@V matmuls are deferred (ptc tiles buffered) until
    v_sb lands. Later blocks run a lag-3 score->exp->attn@V pipeline.
  - BN rstd uses a DVE Newton iteration (bit-trick seed) instead of
    ScalarE Ln/Exp: the act-table loader thrashes sets otherwise (Ln
    and Exp resolve to different table sets -> 4 extra ~1.3us loads,
    two of them in front of the exp stream).
  - PSUM->SBUF copies (projections) and the output-proj bias add run on
    VectorE, keeping ScalarE exp-only.
  - optional: a fraction of exp tiles can run on VectorE via a
    Schraudolph bf16-bit-trick tensor_scalar (DVE_EXP_EVERY).
"""

import os

import numpy as np

B, L, C = 8, 1024, 256
H = 8
D = 32  # head dim
IMG = 32  # h = w = 32
PAD = 34  # padded image side
EPS = 1e-5
ATT_SCALE = float(C) ** -0.5  # 1/16

# Schraudolph exp in bf16-bits: exp(s*x) ~= bitcast_bf16(i16(x*SCH_A + SCH_B))
SCH_A = (128.0 / float(np.log(2.0))) * ATT_SCALE
SCH_B = 128.0 * (127.0 - 0.0450466)

_CACHE = {}
DEBUG = False
VARIANT = "full"  # "full" | "noattn" | "convonly" (phase timing builds)
SIM_NO_CC = False  # replace AllReduce with local DMA copy (TimelineSim only)
RSTD_MODE = "newton"  # "newton" (DVE) | "lnexp" (ScalarE tables)
# 0 = off; n>0: every nth exp tile on DVE (Schraudolph)
DVE_EXP_EVERY = int(os.environ.get("DVE_EXP_EVERY", "0"))


def _build_nc(repeat=1):
    import concourse.bacc as bacc
    import concourse.tile as tile
    from concourse import mybir

    f32 = mybir.dt.float32
    f32r = mybir.dt.float32r
    bf16 = mybir.dt.bfloat16
    i16 = mybir.dt.int16
    i32 = mybir.dt.int32
    AF = mybir.ActivationFunctionType
    ALU = mybir.AluOpType

    nc = bacc.Bacc(None, target_bir_lowering=False)
    nc.num_devices = 8

    # ---- DRAM parameters (host-prepped layouts) ----
    xt = nc.declare_dram_parameter("xt", [C, PAD * PAD], f32r, isOutput=False)
    yt = nc.declare_dram_parameter("yt", [C, PAD * PAD], f32r, isOutput=False)
    # conv weights: [9(kpos), 2(ci), 2(co), 128, 128]
    wcq = nc.declare_dram_parameter("wcq", [9, 2, 2, 128, 128], f32r, isOutput=False)
    wck = nc.declare_dram_parameter("wck", [9, 2, 2, 128, 128], f32r, isOutput=False)
    wcv = nc.declare_dram_parameter("wcv", [9, 2, 2, 128, 128], f32r, isOutput=False)
    # projection weights W.T tiled: [2(ci), 128, 256(co)]
    pq = nc.declare_dram_parameter("pq", [2, 128, C], f32r, isOutput=False)
    pk = nc.declare_dram_parameter("pk", [2, 128, C], f32r, isOutput=False)
    pv = nc.declare_dram_parameter("pv", [2, 128, C], f32r, isOutput=False)
    po = nc.declare_dram_parameter("po", [2, 128, C], f32r, isOutput=False)
    # gamma/beta pack [128, 12]: cols 0-5 gamma, 6-11 beta
    gb = nc.declare_dram_parameter("gb", [128, 12], f32, isOutput=False)
    bo = nc.declare_dram_parameter("bo", [128, 2], f32, isOutput=False)
    out = nc.declare_dram_parameter("out", [C, L], f32, isOutput=True)

    with tile.TileContext(nc) as tc:
        with tc.tile_pool(name="singles", bufs=1) as singles, \
             tc.tile_pool(name="stats", bufs=1) as statsp, \
             tc.tile_pool(name="bnst", bufs=4) as bnstp, \
             tc.tile_pool(name="rep", bufs=2) as repp, \
             tc.tile_pool(name="pt", bufs=20) as ptp, \
             tc.tile_pool(name="ps", bufs=2, space="PSUM") as psp, \
             tc.tile_pool(name="score_ps", bufs=3, space="PSUM") as scorep, \
             tc.tile_pool(name="dram", bufs=1, space="DRAM") as dramp:

            for _rep in range(repeat):
                # ---------- constants / small tiles ----------
                ones32 = singles.tile([128, 32], bf16)
                nc.vector.memset(ones32[:], 1.0)
                epst = singles.tile([128, 1], f32)
                nc.vector.memset(epst[:], EPS)
                magict = singles.tile([128, 6], f32)
                # f32 whose bits are 0x5f375a86 (Newton-rsqrt magic)
                nc.vector.memset(magict[:], 1.3212019791402893e19)
                gbt = singles.tile([128, 12], f32)
                nc.sync.dma_start(out=gbt[:], in_=gb[:])
                bot = singles.tile([128, 2], f32)
                nc.sync.dma_start(out=bot[:], in_=bo[:])

                # ---------- padded images + weights ----------
                pad_x = singles.tile([128, 2, PAD, PAD], f32r)
                pad_y = singles.tile([128, 2, PAD, PAD], f32r)
                wq_sb = singles.tile([128, 36 * 128], f32r)
                wk_sb = singles.tile([128, 36 * 128], f32r)
                wv_sb = singles.tile([128, 36 * 128], f32r)
                pq_sb = singles.tile([128, 2 * C], f32r)
                pk_sb = singles.tile([128, 2 * C], f32r)
                pv_sb = singles.tile([128, 2 * C], f32r)
                po_sb = singles.tile([128, 2 * C], f32r)

                ytr = yt.rearrange("(c p) m -> p c m", p=128)
                xtr = xt.rearrange("(c p) m -> p c m", p=128)
                wk4 = wk_sb[:].rearrange("p (a t f) -> p a t f", a=3, f=128)
                wq4 = wq_sb[:].rearrange("p (a t f) -> p a t f", a=3, f=128)
                wv4 = wv_sb[:].rearrange("p (a t f) -> p a t f", a=3, f=128)
                wckr = wck.rearrange("(a g) b c p f -> p a (g b c) f", a=3)
                wcqr = wcq.rearrange("(a g) b c p f -> p a (g b c) f", a=3)
                wcvr = wcv.rearrange("(a g) b c p f -> p a (g b c) f", a=3)
                # two HWDGE rings, FIFO each; emit in consumption order:
                # sync: pad_y -> wcq -> pk -> pv ; scalar: wck -> pad_x -> pq
                # -> wcv -> po.  conv_k needs pad_y(sync)+wck(scalar) ~4.5us.
                for ci in range(2):
                    nc.sync.dma_start(out=pad_y[:, ci], in_=ytr[:, ci])
                for a in range(3):
                    nc.scalar.dma_start(out=wk4[:, a], in_=wckr[:, a])
                for a in range(3):
                    nc.sync.dma_start(out=wq4[:, a], in_=wcqr[:, a])
                for ci in range(2):
                    nc.scalar.dma_start(out=pad_x[:, ci], in_=xtr[:, ci])
                for eng, psb, pdr in ((nc.sync, pk_sb, pk), (nc.scalar, pq_sb, pq),
                                      (nc.sync, pv_sb, pv)):
                    eng.dma_start(
                        out=psb[:].rearrange("p (t f) -> p t f", f=C),
                        in_=pdr.rearrange("t p f -> p t f"),
                    )
                for a in range(3):
                    nc.scalar.dma_start(out=wv4[:, a], in_=wcvr[:, a])
                nc.scalar.dma_start(
                    out=po_sb[:].rearrange("p (t f) -> p t f", f=C),
                    in_=po.rearrange("t p f -> p t f"),
                )

                # ---------- conv machinery ----------
                kraw = singles.tile([128, 2 * L], f32)
                vraw = singles.tile([128, 2 * L], f32)
                qraw = singles.tile([128, 2 * L], f32)
                st = statsp.tile([128, 12], f32)  # local (mean, m2) pairs

                def conv_quarter(pad_t, w_sb, raw, stat_base, co, half):
                    ps = psp.tile([128, 512], f32, tag="ps")
                    idx = 0
                    for kp in range(9):
                        ky, kx = kp // 3, kp % 3
                        for ci in range(2):
                            blk = (kp * 2 + ci) * 2 + co
                            lhsT = w_sb[:, blk * 128:(blk + 1) * 128]
                            rhs = pad_t[:, ci, ky + half * 16: ky + half * 16 + 16,
                                        kx: kx + 32]
                            nc.tensor.matmul(ps[:], lhsT, rhs,
                                             start=(idx == 0), stop=(idx == 17))
                            idx += 1
                    nc.vector.tensor_copy(
                        out=raw[:, co * L + half * 512: co * L + (half + 1) * 512].bitcast(f32r),
                        in_=ps[:])
                    if half == 1:
                        k = stat_base + co
                        st6 = bnstp.tile([128, 2, 6], f32, tag="st6")
                        nc.vector.bn_stats(st6[:, 0, :], raw[:, co * L: co * L + 512])
                        nc.vector.bn_stats(st6[:, 1, :], raw[:, co * L + 512: co * L + 1024])
                        nc.vector.bn_aggr(st[:, 2 * k: 2 * k + 2], st6[:])
                        # m2 = mean^2 + var (in place on the var column)
                        nc.vector.scalar_tensor_tensor(
                            out=st[:, 2 * k + 1: 2 * k + 2],
                            in0=st[:, 2 * k: 2 * k + 1],
                            scalar=st[:, 2 * k: 2 * k + 1],
                            in1=st[:, 2 * k + 1: 2 * k + 2],
                            op0=ALU.mult, op1=ALU.add,
                        )

                def conv_full(pad_t, w_sb, raw, stat_base):
                    for co in range(2):
                        for half in range(2):
                            conv_quarter(pad_t, w_sb, raw, stat_base, co, half)

                conv_full(pad_y, wk_sb, kraw, 2)
                conv_full(pad_x, wq_sb, qraw, 0)

                # ---------- AllReduce #1: q+k stats ----------
                cc_in1 = dramp.tile([128, 8], f32)
                cc_out1 = dramp.tile([128, 8], f32)
                nc.sync.dma_start(out=cc_in1[:], in_=st[:, 0:8])
                if SIM_NO_CC:
                    nc.gpsimd.dma_start(out=cc_out1[:], in_=cc_in1[:])
                else:
                    nc.gpsimd.collective_compute(
                        "AllReduce", ALU.add,
                        replica_groups=[list(range(8))],
                        ins=[cc_in1[:].opt()], outs=[cc_out1[:].opt()],
                    )
                gstats = statsp.tile([128, 12], f32)
                nc.sync.dma_start(out=gstats[:, 0:8], in_=cc_out1[:])

                # first quarter of conv_v covers the CC1 latency
                if VARIANT == "full":
                    conv_quarter(pad_y, wv_sb, vraw, 4, 0, 0)
                else:
                    conv_full(pad_y, wv_sb, vraw, 4)

                # ---------- global scale/shift ----------
                var_t = statsp.tile([128, 6], f32)
                nwt_h = statsp.tile([128, 6], f32)
                nwt_y = statsp.tile([128, 6], f32)
                nwt_t = statsp.tile([128, 6], f32)
                scale_t = statsp.tile([128, 6], f32)
                shift_t = statsp.tile([128, 6], f32)

                def bn_post(k0, nk):
                    seg = gstats[:, 2 * k0: 2 * (k0 + nk)]
                    nc.vector.tensor_scalar_mul(seg, seg, 1.0 / 8.0)
                    g2 = seg.rearrange("p (k two) -> p k two", two=2)
                    gmean = g2[:, :, 0]
                    gm2 = g2[:, :, 1]
                    vt = var_t[:, k0: k0 + nk]
                    nc.vector.tensor_mul(vt, gmean, gmean)
                    nc.vector.tensor_sub(vt, gm2, vt)
                    if RSTD_MODE == "newton":
                        # rstd = rsqrt(var+eps): bit-trick seed + 2 Newton steps
                        nc.vector.tensor_scalar_add(vt, vt, EPS)
                        hv = nwt_h[:, k0: k0 + nk]
                        nc.vector.tensor_scalar_mul(hv, vt, 0.5)
                        yv = nwt_y[:, k0: k0 + nk]
                        nc.vector.tensor_scalar(
                            out=yv.bitcast(i32), in0=vt.bitcast(i32),
                            scalar1=1, scalar2=None, op0=ALU.logical_shift_right)
                        nc.vector.tensor_sub(
                            yv.bitcast(i32), magict[:, k0: k0 + nk].bitcast(i32),
                            yv.bitcast(i32))
                        tv = nwt_t[:, k0: k0 + nk]
                        for _ in range(2):
                            nc.vector.tensor_mul(tv, yv, yv)
                            nc.vector.tensor_mul(tv, tv, hv)
                            nc.vector.tensor_scalar(
                                out=tv, in0=tv, scalar1=-1.0, scalar2=1.5,
                                op0=ALU.mult, op1=ALU.add)
                            nc.vector.tensor_mul(yv, yv, tv)
                        vt = yv
                    else:
                        nc.scalar.activation(vt, vt, AF.Ln, bias=epst[:, 0:1], scale=1.0)
                        nc.scalar.activation(vt, vt, AF.Exp, scale=-0.5)
                    sc = scale_t[:, k0: k0 + nk]
                    sh = shift_t[:, k0: k0 + nk]
                    nc.vector.tensor_mul(sc, vt, gbt[:, k0: k0 + nk])
                    nc.vector.tensor_mul(sh, gmean, sc)
                    nc.vector.tensor_sub(sh, gbt[:, 6 + k0: 6 + k0 + nk], sh)

                def bn_apply(raw, base):
                    for ch in range(2):
                        k = base + ch
                        nc.vector.tensor_scalar(
                            out=raw[:, ch * L:(ch + 1) * L].bitcast(f32r),
                            in0=raw[:, ch * L:(ch + 1) * L],
                            scalar1=scale_t[:, k: k + 1],
                            scalar2=shift_t[:, k: k + 1],
                            op0=ALU.mult, op1=ALU.add,
                        )

                bn_post(0, 4)   # q, k
                bn_apply(qraw, 0)
                bn_apply(kraw, 2)

                if VARIANT == "convonly":
                    cc_in2 = dramp.tile([128, 4], f32)
                    cc_out2 = dramp.tile([128, 4], f32)
                    nc.sync.dma_start(out=cc_in2[:], in_=st[:, 8:12])
                    if SIM_NO_CC:
                        nc.gpsimd.dma_start(out=cc_out2[:], in_=cc_in2[:])
                    else:
                        nc.gpsimd.collective_compute(
                            "AllReduce", ALU.add,
                            replica_groups=[list(range(8))],
                            ins=[cc_in2[:].opt()], outs=[cc_out2[:].opt()],
                        )
                    nc.sync.dma_start(out=gstats[:, 8:12], in_=cc_out2[:])
                    bn_post(4, 2)
                    bn_apply(vraw, 4)
                    nc.sync.dma_start(
                        out=out.rearrange("(c p) l -> p c l", p=128),
                        in_=kraw[:].rearrange("p (c l) -> p c l", l=L))
                    continue

                # ---------- q/k projections -> transposed [c, L] ----------
                qT = singles.tile([128, 2 * L], f32)
                kT = singles.tile([128, 2 * L], f32)

                def proj_T(src_t, wsb, dst, co):
                    for lh in range(2):
                        ps = psp.tile([128, 512], f32, tag="ps")
                        for ci in range(2):
                            lhsT = wsb[:, ci * C + co * 128: ci * C + (co + 1) * 128]
                            rhs = src_t[:, ci * L + lh * 512: ci * L + (lh + 1) * 512].bitcast(f32r)
                            nc.tensor.matmul(ps[:], lhsT, rhs,
                                             start=(ci == 0), stop=(ci == 1))
                        nc.vector.tensor_copy(
                            out=dst[:, co * L + lh * 512: co * L + (lh + 1) * 512].bitcast(f32r),
                            in_=ps[:])

                for co in range(2):
                    proj_T(kraw, pk_sb, kT, co)
                    proj_T(qraw, pq_sb, qT, co)

                # ---------- v path helpers (emitted later, interleaved) ----
                v_sb = singles.tile([128, 8 * C], bf16)  # col = tc*256 + co

                def emit_cc2_bn_v():
                    cc_in2 = dramp.tile([128, 4], f32)
                    cc_out2 = dramp.tile([128, 4], f32)
                    nc.sync.dma_start(out=cc_in2[:], in_=st[:, 8:12])
                    if SIM_NO_CC:
                        nc.gpsimd.dma_start(out=cc_out2[:], in_=cc_in2[:])
                    else:
                        nc.gpsimd.collective_compute(
                            "AllReduce", ALU.add,
                            replica_groups=[list(range(8))],
                            ins=[cc_in2[:].opt()], outs=[cc_out2[:].opt()],
                        )
                    nc.sync.dma_start(out=gstats[:, 8:12], in_=cc_out2[:])
                    bn_post(4, 2)
                    bn_apply(vraw, 4)

                def emit_vproj():
                    for lt in range(8):
                        ps = psp.tile([128, C], f32, tag="ps")
                        for ci in range(2):
                            lhsT = vraw[:, ci * L + lt * 128: ci * L + (lt + 1) * 128].bitcast(f32r)
                            rhs = pv_sb[:, ci * C:(ci + 1) * C]
                            nc.tensor.matmul(ps[:], lhsT, rhs, start=(ci == 0), stop=(ci == 1))
                        nc.vector.tensor_copy(out=v_sb[:, lt * C:(lt + 1) * C], in_=ps[:])

                if VARIANT == "noattn":
                    emit_cc2_bn_v()
                    emit_vproj()
                    nc.sync.dma_start(
                        out=out.rearrange("(c p) l -> p c l", p=128),
                        in_=qT[:].rearrange("p (c l) -> p c l", l=L))
                    continue

                # ---------- attention ----------
                attn_oT = singles.tile([128, 2 * L], f32)  # col = g*1024 + l
                exp_ctr = [0]

                def sc_unit(g, lh, tc_i, jp):
                    score = scorep.tile([128, 1024], f32, tag="score")
                    for jj in range(2):
                        j = 2 * jp + jj
                        lhsT = kT[32 * j: 32 * j + 32,
                                  g * L + tc_i * 128: g * L + (tc_i + 1) * 128].bitcast(f32r)
                        rhs = qT[32 * j: 32 * j + 32,
                                 g * L + lh * 512: g * L + (lh + 1) * 512].bitcast(f32r)
                        nc.tensor.matmul(score[:, jj * 512:(jj + 1) * 512],
                                         lhsT, rhs, start=True, stop=True,
                                         tile_position=(32 * j, 0))
                    return score

                def exp_unit(score):
                    ptc = ptp.tile([128, 1024], bf16, tag="pt")
                    exp_ctr[0] += 1
                    if DVE_EXP_EVERY and exp_ctr[0] % DVE_EXP_EVERY == 0:
                        nc.vector.tensor_scalar(
                            out=ptc[:].bitcast(i16), in0=score[:],
                            scalar1=SCH_A, scalar2=SCH_B,
                            op0=ALU.mult, op1=ALU.add)
                    else:
                        nc.scalar.activation(ptc[:], score[:], AF.Exp, scale=ATT_SCALE)
                    return ptc

                def av_unit(av, den, ptc, g, tc_i, jp):
                    for jj in range(2):
                        j = 2 * jp + jj
                        rhs_pt = ptc[:, jj * 512:(jj + 1) * 512]
                        lhsT_v = v_sb[:, tc_i * C + g * 128 + j * 32:
                                      tc_i * C + g * 128 + (j + 1) * 32]
                        nc.tensor.matmul(av[32 * j: 32 * j + 32, :], lhsT_v, rhs_pt,
                                         start=False, stop=False,
                                         tile_position=(0, 32 * j),
                                         skip_group_check=True)
                    for jj in range(2):
                        j = 2 * jp + jj
                        rhs_pt = ptc[:, jj * 512:(jj + 1) * 512]
                        nc.tensor.matmul(den[32 * j: 32 * j + 32, :], ones32[:], rhs_pt,
                                         start=False, stop=False,
                                         tile_position=(0, 32 * j),
                                         skip_group_check=True)

                def new_avden():
                    av = psp.tile([128, 512], f32, tag="ps")
                    den = psp.tile([128, 512], f32, tag="ps")
                    nc.vector.memset(av[:], 0.0)
                    nc.vector.memset(den[:], 0.0)
                    return av, den

                def norm_block(av, den, g, lh):
                    rep = repp.tile([128, 512], f32, tag="rep")
                    nc.vector.reciprocal_approx_fast(out=rep[:], in_=den[:])
                    nc.vector.tensor_mul(
                        attn_oT[:, g * L + lh * 512: g * L + (lh + 1) * 512].bitcast(f32r),
                        av[:], rep[:])

                units = [(t, jp) for t in range(8) for jp in range(2)]

                # --- block (g=0, lh=0): exps buffered, attn@V deferred ---
                # conv_v quarters (co,half) = (0,1),(1,0),(1,1) slot between
                # units so the PE keeps pace with the ScalarE exp stream.
                fillers = {
                    4: lambda: conv_quarter(pad_y, wv_sb, vraw, 4, 0, 1),
                    9: lambda: conv_quarter(pad_y, wv_sb, vraw, 4, 1, 0),
                    13: lambda: conv_quarter(pad_y, wv_sb, vraw, 4, 1, 1),
                }
                backlog = []
                for u, (t, jp) in enumerate(units):
                    score = sc_unit(0, 0, t, jp)
                    backlog.append((exp_unit(score), t, jp))
                    if u in fillers:
                        fillers[u]()
                emit_cc2_bn_v()
                # keep the exp stream fed while v catches up
                lead = [(exp_unit(sc_unit(0, 1, t, jp)), t, jp)
                        for (t, jp) in units[:2]]
                emit_vproj()
                av0, den0 = new_avden()
                for ptc, t, jp in backlog:
                    av_unit(av0, den0, ptc, 0, t, jp)
                norm_block(av0, den0, 0, 0)

                # --- remaining blocks: lag-3 pipeline ---
                def run_block(g, lh, pending):
                    av, den = new_avden()
                    for (t, jp) in units[len(pending):]:
                        score = sc_unit(g, lh, t, jp)
                        pending.append((exp_unit(score), t, jp))
                        if len(pending) > 3:
                            ptc, pt_, pjp = pending.pop(0)
                            av_unit(av, den, ptc, g, pt_, pjp)
                    while pending:
                        ptc, pt_, pjp = pending.pop(0)
                        av_unit(av, den, ptc, g, pt_, pjp)
                    norm_block(av, den, g, lh)

                run_block(0, 1, lead)
                run_block(1, 0, [])
                run_block(1, 1, [])

                # ---------- output projection (transposed) + bias ----------
                out_sb = singles.tile([128, 2 * L], f32)
                for lh in range(2):
                    for co in range(2):
                        ps = psp.tile([128, 512], f32, tag="ps")
                        for ci in range(2):
                            lhsT = po_sb[:, ci * C + co * 128: ci * C + (co + 1) * 128]
                            rhs = attn_oT[:, ci * L + lh * 512: ci * L + (lh + 1) * 512].bitcast(f32r)
                            nc.tensor.matmul(ps[:], lhsT, rhs, start=(ci == 0), stop=(ci == 1))
                        nc.vector.tensor_scalar(
                            out=out_sb[:, co * L + lh * 512: co * L + (lh + 1) * 512],
                            in0=ps[:], scalar1=bot[:, co: co + 1], scalar2=None,
                            op0=ALU.add)

                outr = out.rearrange("(c p) l -> p c l", p=128)
                osr = out_sb[:].rearrange("p (c l) -> p c l", l=L)
                for lh in range(2):
                    nc.sync.dma_start(out=outr[:, :, lh * 512:(lh + 1) * 512],
                                      in_=osr[:, :, lh * 512:(lh + 1) * 512])

    nc.compile()
    return nc


def _prep_weights(conv_q_w, conv_k_w, conv_v_w, Wq, Wk, Wv, Wo,
                  bn_q_g, bn_q_b, bn_k_g, bn_k_b, bn_v_g, bn_v_b, bo):
    def conv_tiles(w):
        # [co, ci, ky, kx] -> [9, 2(ci), 2(co), 128, 128]
        t = np.ascontiguousarray(np.transpose(np.asarray(w, np.float32), (2, 3, 1, 0)))
        t = t.reshape(3, 3, 2, 128, 2, 128).transpose(0, 1, 2, 4, 3, 5)
        return np.ascontiguousarray(t.reshape(9, 2, 2, 128, 128))

    def proj_tiles(w):
        return np.ascontiguousarray(
            np.asarray(w, np.float32).T.reshape(2, 128, C))

    gbp = np.zeros((128, 12), np.float32)
    for i, (g, b) in enumerate(((bn_q_g, bn_q_b), (bn_k_g, bn_k_b), (bn_v_g, bn_v_b))):
        g = np.asarray(g, np.float32).reshape(2, 128)
        b = np.asarray(b, np.float32).reshape(2, 128)
        for ch in range(2):
            gbp[:, 2 * i + ch] = g[ch]
            gbp[:, 6 + 2 * i + ch] = b[ch]
    bop = np.ascontiguousarray(np.asarray(bo, np.float32).reshape(2, 128).T)
    return {
        "wcq": conv_tiles(conv_q_w), "wck": conv_tiles(conv_k_w),
        "wcv": conv_tiles(conv_v_w),
        "pq": proj_tiles(Wq), "pk": proj_tiles(Wk), "pv": proj_tiles(Wv),
        "po": proj_tiles(Wo),
        "gb": gbp, "bo": bop,
    }


def _get_nc(repeat=1):
    key = ("nc", repeat, VARIANT, DEBUG, RSTD_MODE, DVE_EXP_EVERY)
    if key not in _CACHE:
        _CACHE[key] = _build_nc(repeat)
    return _CACHE[key]


def run_spmd(in_maps, repeat=1, **kw):
    from concourse.bass_utils import run_bass_kernel_spmd
    return run_bass_kernel_spmd(_get_nc(repeat), in_maps, list(range(8)), **kw)


def _get_executor(repeat=1):
    """Build the sharded jitted callable once (mirrors
    bass2jax.run_bass_via_pjrt's multi-core path) so repeated calls skip
    retracing/compilation."""
    key = ("exec", repeat, VARIANT, RSTD_MODE, DVE_EXP_EVERY)
    if key in _CACHE:
        return _CACHE[key]
    import jax
    import numpy as _np
    from jax.sharding import Mesh, PartitionSpec
    from jax.experimental.shard_map import shard_map
    from concourse import bass2jax, mybir

    nc = _get_nc(repeat)
    bass2jax.install_neuronx_cc_hook()
    partition_name = nc.partition_id_tensor.name if nc.partition_id_tensor else None

    in_names, out_names, out_avals, zero_outs = [], [], [], []
    for alloc in nc.m.functions[0].allocations:
        if not isinstance(alloc, mybir.MemoryLocationSet):
            continue
        name = alloc.memorylocations[0].name
        if alloc.kind == "ExternalInput":
            if name != partition_name:
                in_names.append(name)
        elif alloc.kind == "ExternalOutput":
            dt_np = mybir.dt.np(alloc.dtype)
            shape = tuple(alloc.tensor_shape)
            out_avals.append(jax.core.ShapedArray(shape, dt_np))
            out_names.append(name)
            zero_outs.append(_np.zeros(shape, dt_np))

    n_params = len(in_names)
    n_outs = len(out_names)
    all_in_names = list(in_names) + list(out_names)
    if partition_name is not None:
        all_in_names.append(partition_name)
    donate = tuple(range(n_params, n_params + n_outs))

    def _body(*args):
        operands = list(args)
        if partition_name is not None:
            operands.append(bass2jax.partition_id_tensor())
        outs = bass2jax._bass_exec_p.bind(
            *operands,
            out_avals=tuple(out_avals),
            in_names=tuple(all_in_names),
            out_names=tuple(out_names),
            lowering_input_output_aliases=(),
            sim_require_finite=True,
            sim_require_nnan=True,
            nc=nc,
        )
        return tuple(outs)

    devices = jax.devices()[:B]
    mesh = Mesh(np.asarray(devices), ("core",))
    in_specs = (PartitionSpec("core"),) * (n_params + n_outs)
    out_specs = (PartitionSpec("core"),) * n_outs
    sharded = jax.jit(
        shard_map(_body, mesh=mesh, in_specs=in_specs, out_specs=out_specs,
                  check_rep=False),
        donate_argnums=donate, keep_unused=True,
    )
    _CACHE[("mesh", repeat, VARIANT)] = mesh
    _CACHE[("jit", repeat, VARIANT)] = sharded

    def run(in_maps):
        concat_in = [
            np.concatenate([np.asarray(in_maps[c][k]) for c in range(B)], axis=0)
            for k in in_names
        ]
        concat_zeros = [np.zeros((B * z.shape[0], *z.shape[1:]), z.dtype)
                        for z in zero_outs]
        out_arrs = sharded(*concat_in, *concat_zeros)
        return out_arrs, out_names, out_avals

    _CACHE[key] = run
    return run


def run_fast(in_maps, repeat=1):
    """Execute via the cached jitted callable; returns per-core dict list."""
    run = _get_executor(repeat)
    out_arrs, out_names, out_avals = run(in_maps)
    return [
        {name: np.asarray(out_arrs[i]).reshape(B, *out_avals[i].shape)[c]
         for i, name in enumerate(out_names)}
        for c in range(B)
    ]


def bench_wall(in_maps, repeat, n_iter):
    """Dispatch n_iter executions of the repeat-R NEFF with device-resident
    inputs and pre-staged donated zero buffers; return total wall seconds."""
    import time as _time
    import jax
    from jax.sharding import NamedSharding, PartitionSpec

    _get_executor(repeat)  # ensure built
    nc = _get_nc(repeat)
    from concourse import mybir
    partition_name = nc.partition_id_tensor.name if nc.partition_id_tensor else None
    in_names, out_shapes = [], []
    for alloc in nc.m.functions[0].allocations:
        if not isinstance(alloc, mybir.MemoryLocationSet):
            continue
        name = alloc.memorylocations[0].name
        if alloc.kind == "ExternalInput" and name != partition_name:
            in_names.append(name)
        elif alloc.kind == "ExternalOutput":
            out_shapes.append((tuple(alloc.tensor_shape), mybir.dt.np(alloc.dtype)))

    key = ("bench_in", repeat, VARIANT)
    if key not in _CACHE:
        mesh = _CACHE[("mesh", repeat, VARIANT)]
        sh = NamedSharding(mesh, PartitionSpec("core"))
        dev_in = [
            jax.device_put(
                np.concatenate([np.asarray(in_maps[c][k]) for c in range(B)], 0), sh)
            for k in in_names
        ]
        _CACHE[key] = (dev_in, sh)
    dev_in, sh = _CACHE[key]

    sharded = _CACHE[("jit", repeat, VARIANT)]
    zero_sets = []
    for _ in range(n_iter):
        zs = [jax.device_put(np.zeros((B * s[0], *s[1:]), dt), sh)
              for (s, dt) in out_shapes]
        zero_sets.append(zs)
    for zs in zero_sets:
        for z in zs:
            z.block_until_ready()

    outs = []
    t0 = _time.perf_counter()
    for it in range(n_iter):
        outs.append(sharded(*dev_in, *zero_sets[it]))
    for o in outs[-1]:
        o.block_until_ready()
    t1 = _time.perf_counter()
    return t1 - t0


def make_in_maps(x, y, h, w, conv_q_w, bn_q_g, bn_q_b,
                 conv_k_w, bn_k_g, bn_k_b, conv_v_w, bn_v_g, bn_v_b,
                 Wq, Wk, Wv, Wo, bo):
    assert int(h) == IMG and int(w) == IMG
    x = np.asarray(x, np.float32)
    y = np.asarray(y, np.float32)
    wmap = _prep_weights(conv_q_w, conv_k_w, conv_v_w, Wq, Wk, Wv, Wo,
                         bn_q_g, bn_q_b, bn_k_g, bn_k_b, bn_v_g, bn_v_b, bo)
    def pad_t(a):
        # [B, L, C] -> [B, C, 34*34] with zero border baked in
        at = np.transpose(a, (0, 2, 1)).reshape(B, C, IMG, IMG)
        ap = np.zeros((B, C, PAD, PAD), np.float32)
        ap[:, :, 1:33, 1:33] = at
        return ap.reshape(B, C, PAD * PAD)

    xT = pad_t(x)
    yT = pad_t(y)
    return [dict(wmap, xt=xT[b], yt=yT[b]) for b in range(B)]


def kernel(**inputs):
    in_maps = make_in_maps(**inputs)
    res = run_fast(in_maps)
    outs = [res[b]["out"] for b in range(B)]  # each [C, L]
    return np.ascontiguousarray(
        np.stack(outs, axis=0).transpose(0, 2, 1)).astype(np.float32)


# revision 9
# speedup vs baseline: 9.4596x; 2.9012x over previous
"""Distributed Trainium2 (Bass/Tile) kernel for nn_Attention_2D.

Pipeline (per batch element): 3x3 conv + BatchNorm (batch stats!) for
Q (from x), K, V (from y) -> linear projections -> multi-head attention
(scale = C**-0.5) -> output projection.

Sharding: data-parallel over batch B=8 across the 8 NeuronCores (one
image per core). The only cross-core dependency is the BatchNorm
mean/var over the whole batch -> tiny [128,{8,4}] AllReduces.

v2 schedule notes (the baseline staged scores through SBUF via VectorE
copies - 87us of DVE time - and ran conv_v before the projections,
pushing the 73us ScalarE exp stream to start ~50us in):
  - scores stay in PSUM ([128,1024] tiles, 2 banks each, 3 bufs) and
    ScalarE exps them PSUM->SBUF bf16 directly; no staging copies.
  - the exp stream is the attention bottleneck (64 x ~1.15us), so the
    emission order is built around starting it as early as possible and
    never starving it: conv_k, conv_q, CC1 (stats) covered by one
    quarter of conv_v, BN, q/k projections, then attention group
    (g=0,lh=0) begins; the remaining 3 quarters of conv_v + CC2 + the
    v projection are interleaved between score/exp units of that first
    block, whose attn@V matmuls are deferred (ptc tiles buffered) until
    v_sb lands. Later blocks run a lag-3 score->exp->attn@V pipeline.
  - BN rstd uses a DVE Newton iteration (bit-trick seed) instead of
    ScalarE Ln/Exp: the act-table loader thrashes sets otherwise (Ln
    and Exp resolve to different table sets -> 4 extra ~1.3us loads,
    two of them in front of the exp stream).
  - PSUM->SBUF copies (projections) and the output-proj bias add run on
    VectorE, keeping ScalarE exp-only.
  - optional: a fraction of exp tiles can run on VectorE via a
    Schraudolph bf16-bit-trick tensor_scalar (DVE_EXP_EVERY).
"""

import os

import numpy as np

B, L, C = 8, 1024, 256
H = 8
D = 32  # head dim
IMG = 32  # h = w = 32
PAD = 34  # padded image side
EPS = 1e-5
ATT_SCALE = float(C) ** -0.5  # 1/16

# Schraudolph exp in bf16-bits: exp(s*x) ~= bitcast_bf16(i16(x*SCH_A + SCH_B))
SCH_A = (128.0 / float(np.log(2.0))) * ATT_SCALE
SCH_B = 128.0 * (127.0 - 0.0450466)

_CACHE = {}
DEBUG = False
VARIANT = "full"  # "full" | "noattn" | "convonly" (phase timing builds)
SIM_NO_CC = False  # replace AllReduce with local DMA copy (TimelineSim only)
RSTD_MODE = "newton"  # "newton" (DVE) | "lnexp" (ScalarE tables)
# 0 = off; n>0: every nth exp tile on DVE (Schraudolph)
DVE_EXP_EVERY = int(os.environ.get("DVE_EXP_EVERY", "0"))


def _build_nc(repeat=1):
    import concourse.bacc as bacc
    import concourse.tile as tile
    from concourse import mybir

    f32 = mybir.dt.float32
    f32r = mybir.dt.float32r
    bf16 = mybir.dt.bfloat16
    i16 = mybir.dt.int16
    i32 = mybir.dt.int32
    AF = mybir.ActivationFunctionType
    ALU = mybir.AluOpType

    nc = bacc.Bacc(None, target_bir_lowering=False)
    nc.num_devices = 8

    # ---- DRAM parameters (host-prepped layouts) ----
    xt = nc.declare_dram_parameter("xt", [C, PAD * PAD], f32r, isOutput=False)
    yt = nc.declare_dram_parameter("yt", [C, PAD * PAD], f32r, isOutput=False)
    # conv weights: [9(kpos), 2(ci), 2(co), 128, 128]
    wcq = nc.declare_dram_parameter("wcq", [9, 2, 2, 128, 128], f32r, isOutput=False)
    wck = nc.declare_dram_parameter("wck", [9, 2, 2, 128, 128], f32r, isOutput=False)
    wcv = nc.declare_dram_parameter("wcv", [9, 2, 2, 128, 128], f32r, isOutput=False)
    # projection weights W.T tiled: [2(ci), 128, 256(co)]
    pq = nc.declare_dram_parameter("pq", [2, 128, C], f32r, isOutput=False)
    pk = nc.declare_dram_parameter("pk", [2, 128, C], f32r, isOutput=False)
    pv = nc.declare_dram_parameter("pv", [2, 128, C], f32r, isOutput=False)
    po = nc.declare_dram_parameter("po", [2, 128, C], f32r, isOutput=False)
    # gamma/beta pack [128, 12]: cols 0-5 gamma, 6-11 beta
    gb = nc.declare_dram_parameter("gb", [128, 12], f32, isOutput=False)
    bo = nc.declare_dram_parameter("bo", [128, 2], f32, isOutput=False)
    out = nc.declare_dram_parameter("out", [C, L], f32, isOutput=True)

    with tile.TileContext(nc) as tc:
        with tc.tile_pool(name="singles", bufs=1) as singles, \
             tc.tile_pool(name="stats", bufs=1) as statsp, \
             tc.tile_pool(name="bnst", bufs=4) as bnstp, \
             tc.tile_pool(name="rep", bufs=2) as repp, \
             tc.tile_pool(name="pt", bufs=20) as ptp, \
             tc.tile_pool(name="ps", bufs=2, space="PSUM") as psp, \
             tc.tile_pool(name="score_ps", bufs=3, space="PSUM") as scorep, \
             tc.tile_pool(name="dram", bufs=1, space="DRAM") as dramp:

            for _rep in range(repeat):
                # ---------- constants / small tiles ----------
                ones32 = singles.tile([128, 32], bf16)
                nc.vector.memset(ones32[:], 1.0)
                epst = singles.tile([128, 1], f32)
                nc.vector.memset(epst[:], EPS)
                magict = singles.tile([128, 6], f32)
                # f32 whose bits are 0x5f375a86 (Newton-rsqrt magic)
                nc.vector.memset(magict[:], 1.3212019791402893e19)
                gbt = singles.tile([128, 12], f32)
                nc.sync.dma_start(out=gbt[:], in_=gb[:])
                bot = singles.tile([128, 2], f32)
                nc.sync.dma_start(out=bot[:], in_=bo[:])

                # ---------- padded images + weights ----------
                pad_x = singles.tile([128, 2, PAD, PAD], f32r)
                pad_y = singles.tile([128, 2, PAD, PAD], f32r)
                wq_sb = singles.tile([128, 36 * 128], f32r)
                wk_sb = singles.tile([128, 36 * 128], f32r)
                wv_sb = singles.tile([128, 36 * 128], f32r)
                pq_sb = singles.tile([128, 2 * C], f32r)
                pk_sb = singles.tile([128, 2 * C], f32r)
                pv_sb = singles.tile([128, 2 * C], f32r)
                po_sb = singles.tile([128, 2 * C], f32r)

                ytr = yt.rearrange("(c p) m -> p c m", p=128)
                xtr = xt.rearrange("(c p) m -> p c m", p=128)
                wk4 = wk_sb[:].rearrange("p (a t f) -> p a t f", a=3, f=128)
                wq4 = wq_sb[:].rearrange("p (a t f) -> p a t f", a=3, f=128)
                wv4 = wv_sb[:].rearrange("p (a t f) -> p a t f", a=3, f=128)
                wckr = wck.rearrange("(a g) b c p f -> p a (g b c) f", a=3)
                wcqr = wcq.rearrange("(a g) b c p f -> p a (g b c) f", a=3)
                wcvr = wcv.rearrange("(a g) b c p f -> p a (g b c) f", a=3)
                # two HWDGE rings, FIFO each; emit in consumption order:
                # sync: pad_y -> wcq -> pk -> pv ; scalar: wck -> pad_x -> pq
                # -> wcv -> po.  conv_k needs pad_y(sync)+wck(scalar) ~4.5us.
                for ci in range(2):
                    nc.sync.dma_start(out=pad_y[:, ci], in_=ytr[:, ci])
                for a in range(3):
                    nc.scalar.dma_start(out=wk4[:, a], in_=wckr[:, a])
                for a in range(3):
                    nc.sync.dma_start(out=wq4[:, a], in_=wcqr[:, a])
                for ci in range(2):
                    nc.scalar.dma_start(out=pad_x[:, ci], in_=xtr[:, ci])
                for eng, psb, pdr in ((nc.sync, pk_sb, pk), (nc.scalar, pq_sb, pq),
                                      (nc.sync, pv_sb, pv)):
                    eng.dma_start(
                        out=psb[:].rearrange("p (t f) -> p t f", f=C),
                        in_=pdr.rearrange("t p f -> p t f"),
                    )
                for a in range(3):
                    nc.scalar.dma_start(out=wv4[:, a], in_=wcvr[:, a])
                nc.scalar.dma_start(
                    out=po_sb[:].rearrange("p (t f) -> p t f", f=C),
                    in_=po.rearrange("t p f -> p t f"),
                )

                # ---------- conv machinery ----------
                kraw = singles.tile([128, 2 * L], f32)
                vraw = singles.tile([128, 2 * L], f32)
                qraw = singles.tile([128, 2 * L], f32)
                st = statsp.tile([128, 12], f32)  # local (mean, m2) pairs

                def conv_quarter(pad_t, w_sb, raw, stat_base, co, half):
                    ps = psp.tile([128, 512], f32, tag="ps")
                    idx = 0
                    for kp in range(9):
                        ky, kx = kp // 3, kp % 3
                        for ci in range(2):
                            blk = (kp * 2 + ci) * 2 + co
                            lhsT = w_sb[:, blk * 128:(blk + 1) * 128]
                            rhs = pad_t[:, ci, ky + half * 16: ky + half * 16 + 16,
                                        kx: kx + 32]
                            nc.tensor.matmul(ps[:], lhsT, rhs,
                                             start=(idx == 0), stop=(idx == 17))
                            idx += 1
                    nc.vector.tensor_copy(
                        out=raw[:, co * L + half * 512: co * L + (half + 1) * 512].bitcast(f32r),
                        in_=ps[:])
                    if half == 1:
                        k = stat_base + co
                        st6 = bnstp.tile([128, 2, 6], f32, tag="st6")
                        nc.vector.bn_stats(st6[:, 0, :], raw[:, co * L: co * L + 512])
                        nc.vector.bn_stats(st6[:, 1, :], raw[:, co * L + 512: co * L + 1024])
                        nc.vector.bn_aggr(st[:, 2 * k: 2 * k + 2], st6[:])
                        # m2 = mean^2 + var (in place on the var column)
                        nc.vector.scalar_tensor_tensor(
                            out=st[:, 2 * k + 1: 2 * k + 2],
                            in0=st[:, 2 * k: 2 * k + 1],
                            scalar=st[:, 2 * k: 2 * k + 1],
                            in1=st[:, 2 * k + 1: 2 * k + 2],
                            op0=ALU.mult, op1=ALU.add,
                        )

                def conv_full(pad_t, w_sb, raw, stat_base):
                    for co in range(2):
                        for half in range(2):
                            conv_quarter(pad_t, w_sb, raw, stat_base, co, half)

                conv_full(pad_y, wk_sb, kraw, 2)
                conv_full(pad_x, wq_sb, qraw, 0)

                # ---------- AllReduce #1: q+k stats ----------
                cc_in1 = dramp.tile([128, 8], f32)
                cc_out1 = dramp.tile([128, 8], f32)
                nc.sync.dma_start(out=cc_in1[:], in_=st[:, 0:8])
                if SIM_NO_CC:
                    nc.gpsimd.dma_start(out=cc_out1[:], in_=cc_in1[:])
                else:
                    nc.gpsimd.collective_compute(
                        "AllReduce", ALU.add,
                        replica_groups=[list(range(8))],
                        ins=[cc_in1[:].opt()], outs=[cc_out1[:].opt()],
                    )
                gstats = statsp.tile([128, 12], f32)
                nc.sync.dma_start(out=gstats[:, 0:8], in_=cc_out1[:])

                # first quarter of conv_v covers the CC1 latency
                if VARIANT == "full":
                    conv_quarter(pad_y, wv_sb, vraw, 4, 0, 0)
                else:
                    conv_full(pad_y, wv_sb, vraw, 4)

                # ---------- global scale/shift ----------
                var_t = statsp.tile([128, 6], f32)
                nwt_h = statsp.tile([128, 6], f32)
                nwt_y = statsp.tile([128, 6], f32)
                nwt_t = statsp.tile([128, 6], f32)
                scale_t = statsp.tile([128, 6], f32)
                shift_t = statsp.tile([128, 6], f32)

                def bn_post(k0, nk):
                    seg = gstats[:, 2 * k0: 2 * (k0 + nk)]
                    nc.vector.tensor_scalar_mul(seg, seg, 1.0 / 8.0)
                    g2 = seg.rearrange("p (k two) -> p k two", two=2)
                    gmean = g2[:, :, 0]
                    gm2 = g2[:, :, 1]
                    vt = var_t[:, k0: k0 + nk]
                    nc.vector.tensor_mul(vt, gmean, gmean)
                    nc.vector.tensor_sub(vt, gm2, vt)
                    if RSTD_MODE == "newton":
                        # rstd = rsqrt(var+eps): bit-trick seed + 2 Newton steps
                        nc.vector.tensor_scalar_add(vt, vt, EPS)
                        hv = nwt_h[:, k0: k0 + nk]
                        nc.vector.tensor_scalar_mul(hv, vt, 0.5)
                        yv = nwt_y[:, k0: k0 + nk]
                        nc.vector.tensor_scalar(
                            out=yv.bitcast(i32), in0=vt.bitcast(i32),
                            scalar1=1, scalar2=None, op0=ALU.logical_shift_right)
                        nc.vector.tensor_sub(
                            yv.bitcast(i32), magict[:, k0: k0 + nk].bitcast(i32),
                            yv.bitcast(i32))
                        tv = nwt_t[:, k0: k0 + nk]
                        for _ in range(2):
                            nc.vector.tensor_mul(tv, yv, yv)
                            nc.vector.tensor_mul(tv, tv, hv)
                            nc.vector.tensor_scalar(
                                out=tv, in0=tv, scalar1=-1.0, scalar2=1.5,
                                op0=ALU.mult, op1=ALU.add)
                            nc.vector.tensor_mul(yv, yv, tv)
                        vt = yv
                    else:
                        nc.scalar.activation(vt, vt, AF.Ln, bias=epst[:, 0:1], scale=1.0)
                        nc.scalar.activation(vt, vt, AF.Exp, scale=-0.5)
                    sc = scale_t[:, k0: k0 + nk]
                    sh = shift_t[:, k0: k0 + nk]
                    nc.vector.tensor_mul(sc, vt, gbt[:, k0: k0 + nk])
                    nc.vector.tensor_mul(sh, gmean, sc)
                    nc.vector.tensor_sub(sh, gbt[:, 6 + k0: 6 + k0 + nk], sh)

                def bn_apply(raw, base):
                    for ch in range(2):
                        k = base + ch
                        nc.vector.tensor_scalar(
                            out=raw[:, ch * L:(ch + 1) * L].bitcast(f32r),
                            in0=raw[:, ch * L:(ch + 1) * L],
                            scalar1=scale_t[:, k: k + 1],
                            scalar2=shift_t[:, k: k + 1],
                            op0=ALU.mult, op1=ALU.add,
                        )

                bn_post(0, 4)   # q, k
                bn_apply(qraw, 0)
                bn_apply(kraw, 2)

                if VARIANT == "convonly":
                    cc_in2 = dramp.tile([128, 4], f32)
                    cc_out2 = dramp.tile([128, 4], f32)
                    nc.sync.dma_start(out=cc_in2[:], in_=st[:, 8:12])
                    if SIM_NO_CC:
                        nc.gpsimd.dma_start(out=cc_out2[:], in_=cc_in2[:])
                    else:
                        nc.gpsimd.collective_compute(
                            "AllReduce", ALU.add,
                            replica_groups=[list(range(8))],
                            ins=[cc_in2[:].opt()], outs=[cc_out2[:].opt()],
                        )
                    nc.sync.dma_start(out=gstats[:, 8:12], in_=cc_out2[:])
                    bn_post(4, 2)
                    bn_apply(vraw, 4)
                    nc.sync.dma_start(
                        out=out.rearrange("(c p) l -> p c l", p=128),
                        in_=kraw[:].rearrange("p (c l) -> p c l", l=L))
                    continue

                # ---------- q/k projections -> transposed [c, L] ----------
                qT = singles.tile([128, 2 * L], f32)
                kT = singles.tile([128, 2 * L], f32)

                def proj_T(src_t, wsb, dst, co):
                    for lh in range(2):
                        ps = psp.tile([128, 512], f32, tag="ps")
                        for ci in range(2):
                            lhsT = wsb[:, ci * C + co * 128: ci * C + (co + 1) * 128]
                            rhs = src_t[:, ci * L + lh * 512: ci * L + (lh + 1) * 512].bitcast(f32r)
                            nc.tensor.matmul(ps[:], lhsT, rhs,
                                             start=(ci == 0), stop=(ci == 1))
                        nc.vector.tensor_copy(
                            out=dst[:, co * L + lh * 512: co * L + (lh + 1) * 512].bitcast(f32r),
                            in_=ps[:])

                for co in range(2):
                    proj_T(kraw, pk_sb, kT, co)
                    proj_T(qraw, pq_sb, qT, co)

                # ---------- v path helpers (emitted later, interleaved) ----
                v_sb = singles.tile([128, 8 * C], bf16)  # col = tc*256 + co

                def emit_cc2_bn_v():
                    cc_in2 = dramp.tile([128, 4], f32)
                    cc_out2 = dramp.tile([128, 4], f32)
                    nc.sync.dma_start(out=cc_in2[:], in_=st[:, 8:12])
                    if SIM_NO_CC:
                        nc.gpsimd.dma_start(out=cc_out2[:], in_=cc_in2[:])
                    else:
                        nc.gpsimd.collective_compute(
                            "AllReduce", ALU.add,
                            replica_groups=[list(range(8))],
                            ins=[cc_in2[:].opt()], outs=[cc_out2[:].opt()],
                        )
                    nc.sync.dma_start(out=gstats[:, 8:12], in_=cc_out2[:])
                    bn_post(4, 2)
                    bn_apply(vraw, 4)

                def emit_vproj():
                    for lt in range(8):
                        ps = psp.tile([128, C], f32, tag="ps")
                        for ci in range(2):
                            lhsT = vraw[:, ci * L + lt * 128: ci * L + (lt + 1) * 128].bitcast(f32r)
                            rhs = pv_sb[:, ci * C:(ci + 1) * C]
                            nc.tensor.matmul(ps[:], lhsT, rhs, start=(ci == 0), stop=(ci == 1))
                        nc.vector.tensor_copy(out=v_sb[:, lt * C:(lt + 1) * C], in_=ps[:])

                if VARIANT == "noattn":
                    emit_cc2_bn_v()
                    emit_vproj()
                    nc.sync.dma_start(
                        out=out.rearrange("(c p) l -> p c l", p=128),
                        in_=qT[:].rearrange("p (c l) -> p c l", l=L))
                    continue

                # ---------- attention ----------
                attn_oT = singles.tile([128, 2 * L], f32)  # col = g*1024 + l
                exp_ctr = [0]

                def sc_unit(g, lh, tc_i, jp):
                    score = scorep.tile([128, 1024], f32, tag="score")
                    for jj in range(2):
                        j = 2 * jp + jj
                        lhsT = kT[32 * j: 32 * j + 32,
                                  g * L + tc_i * 128: g * L + (tc_i + 1) * 128].bitcast(f32r)
                        rhs = qT[32 * j: 32 * j + 32,
                                 g * L + lh * 512: g * L + (lh + 1) * 512].bitcast(f32r)
                        nc.tensor.matmul(score[:, jj * 512:(jj + 1) * 512],
                                         lhsT, rhs, start=True, stop=True,
                                         tile_position=(32 * j, 0))
                    return score

                def exp_unit(score):
                    ptc = ptp.tile([128, 1024], bf16, tag="pt")
                    exp_ctr[0] += 1
                    if DVE_EXP_EVERY and exp_ctr[0] % DVE_EXP_EVERY == 0:
                        nc.vector.tensor_scalar(
                            out=ptc[:].bitcast(i16), in0=score[:],
                            scalar1=SCH_A, scalar2=SCH_B,
                            op0=ALU.mult, op1=ALU.add)
                    else:
                        nc.scalar.activation(ptc[:], score[:], AF.Exp, scale=ATT_SCALE)
                    return ptc

                def av_unit(av, den, ptc, g, tc_i, jp):
                    for jj in range(2):
                        j = 2 * jp + jj
                        rhs_pt = ptc[:, jj * 512:(jj + 1) * 512]
                        lhsT_v = v_sb[:, tc_i * C + g * 128 + j * 32:
                                      tc_i * C + g * 128 + (j + 1) * 32]
                        nc.tensor.matmul(av[32 * j: 32 * j + 32, :], lhsT_v, rhs_pt,
                                         start=False, stop=False,
                                         tile_position=(0, 32 * j),
                                         skip_group_check=True)
                    for jj in range(2):
                        j = 2 * jp + jj
                        rhs_pt = ptc[:, jj * 512:(jj + 1) * 512]
                        nc.tensor.matmul(den[32 * j: 32 * j + 32, :], ones32[:], rhs_pt,
                                         start=False, stop=False,
                                         tile_position=(0, 32 * j),
                                         skip_group_check=True)

                def new_avden():
                    av = psp.tile([128, 512], f32, tag="ps")
                    den = psp.tile([128, 512], f32, tag="ps")
                    nc.vector.memset(av[:], 0.0)
                    nc.vector.memset(den[:], 0.0)
                    return av, den

                def norm_block(av, den, g, lh):
                    rep = repp.tile([128, 512], f32, tag="rep")
                    nc.vector.reciprocal_approx_fast(out=rep[:], in_=den[:])
                    nc.vector.tensor_mul(
                        attn_oT[:, g * L + lh * 512: g * L + (lh + 1) * 512].bitcast(f32r),
                        av[:], rep[:])

                units = [(t, jp) for t in range(8) for jp in range(2)]

                # --- block (g=0, lh=0): exps buffered, attn@V deferred ---
                # conv_v quarters (co,half) = (0,1),(1,0),(1,1) slot between
                # units so the PE keeps pace with the ScalarE exp stream.
                fillers = {
                    4: lambda: conv_quarter(pad_y, wv_sb, vraw, 4, 0, 1),
                    9: lambda: conv_quarter(pad_y, wv_sb, vraw, 4, 1, 0),
                    13: lambda: conv_quarter(pad_y, wv_sb, vraw, 4, 1, 1),
                }
                backlog = []
                for u, (t, jp) in enumerate(units):
                    score = sc_unit(0, 0, t, jp)
                    backlog.append((exp_unit(score), t, jp))
                    if u in fillers:
                        fillers[u]()
                emit_cc2_bn_v()
                # keep the exp stream fed while v catches up
                lead = [(exp_unit(sc_unit(0, 1, t, jp)), t, jp)
                        for (t, jp) in units[:2]]
                emit_vproj()
                av0, den0 = new_avden()
                for ptc, t, jp in backlog:
                    av_unit(av0, den0, ptc, 0, t, jp)
                norm_block(av0, den0, 0, 0)

                # --- remaining blocks: lag-3 pipeline ---
                def run_block(g, lh, pending):
                    av, den = new_avden()
                    for (t, jp) in units[len(pending):]:
                        score = sc_unit(g, lh, t, jp)
                        pending.append((exp_unit(score), t, jp))
                        if len(pending) > 3:
                            ptc, pt_, pjp = pending.pop(0)
                            av_unit(av, den, ptc, g, pt_, pjp)
                    while pending:
                        ptc, pt_, pjp = pending.pop(0)
                        av_unit(av, den, ptc, g, pt_, pjp)
                    norm_block(av, den, g, lh)

                run_block(0, 1, lead)
                run_block(1, 0, [])
                run_block(1, 1, [])

                # ---------- output projection (transposed) + bias ----------
                out_sb = singles.tile([128, 2 * L], f32)
                for lh in range(2):
                    for co in range(2):
                        ps = psp.tile([128, 512], f32, tag="ps")
                        for ci in range(2):
                            lhsT = po_sb[:, ci * C + co * 128: ci * C + (co + 1) * 128]
                            rhs = attn_oT[:, ci * L + lh * 512: ci * L + (lh + 1) * 512].bitcast(f32r)
                            nc.tensor.matmul(ps[:], lhsT, rhs, start=(ci == 0), stop=(ci == 1))
                        nc.vector.tensor_scalar(
                            out=out_sb[:, co * L + lh * 512: co * L + (lh + 1) * 512],
                            in0=ps[:], scalar1=bot[:, co: co + 1], scalar2=None,
                            op0=ALU.add)

                outr = out.rearrange("(c p) l -> p c l", p=128)
                osr = out_sb[:].rearrange("p (c l) -> p c l", l=L)
                for lh in range(2):
                    nc.sync.dma_start(out=outr[:, :, lh * 512:(lh + 1) * 512],
                                      in_=osr[:, :, lh * 512:(lh + 1) * 512])

    nc.compile()
    return nc


def _prep_weights(conv_q_w, conv_k_w, conv_v_w, Wq, Wk, Wv, Wo,
                  bn_q_g, bn_q_b, bn_k_g, bn_k_b, bn_v_g, bn_v_b, bo):
    def conv_tiles(w):
        # [co, ci, ky, kx] -> [9, 2(ci), 2(co), 128, 128]
        t = np.ascontiguousarray(np.transpose(np.asarray(w, np.float32), (2, 3, 1, 0)))
        t = t.reshape(3, 3, 2, 128, 2, 128).transpose(0, 1, 2, 4, 3, 5)
        return np.ascontiguousarray(t.reshape(9, 2, 2, 128, 128))

    def proj_tiles(w):
        return np.ascontiguousarray(
            np.asarray(w, np.float32).T.reshape(2, 128, C))

    gbp = np.zeros((128, 12), np.float32)
    for i, (g, b) in enumerate(((bn_q_g, bn_q_b), (bn_k_g, bn_k_b), (bn_v_g, bn_v_b))):
        g = np.asarray(g, np.float32).reshape(2, 128)
        b = np.asarray(b, np.float32).reshape(2, 128)
        for ch in range(2):
            gbp[:, 2 * i + ch] = g[ch]
            gbp[:, 6 + 2 * i + ch] = b[ch]
    bop = np.ascontiguousarray(np.asarray(bo, np.float32).reshape(2, 128).T)
    return {
        "wcq": conv_tiles(conv_q_w), "wck": conv_tiles(conv_k_w),
        "wcv": conv_tiles(conv_v_w),
        "pq": proj_tiles(Wq), "pk": proj_tiles(Wk), "pv": proj_tiles(Wv),
        "po": proj_tiles(Wo),
        "gb": gbp, "bo": bop,
    }


def _get_nc(repeat=1):
    key = ("nc", repeat, VARIANT, DEBUG, RSTD_MODE, DVE_EXP_EVERY)
    if key not in _CACHE:
        _CACHE[key] = _build_nc(repeat)
    return _CACHE[key]


def run_spmd(in_maps, repeat=1, **kw):
    from concourse.bass_utils import run_bass_kernel_spmd
    return run_bass_kernel_spmd(_get_nc(repeat), in_maps, list(range(8)), **kw)


def _get_executor(repeat=1):
    """Build the sharded jitted callable once (mirrors
    bass2jax.run_bass_via_pjrt's multi-core path) so repeated calls skip
    retracing/compilation."""
    key = ("exec", repeat, VARIANT, RSTD_MODE, DVE_EXP_EVERY)
    if key in _CACHE:
        return _CACHE[key]
    import jax
    import numpy as _np
    from jax.sharding import Mesh, PartitionSpec
    from jax.experimental.shard_map import shard_map
    from concourse import bass2jax, mybir

    nc = _get_nc(repeat)
    bass2jax.install_neuronx_cc_hook()
    partition_name = nc.partition_id_tensor.name if nc.partition_id_tensor else None

    in_names, out_names, out_avals, zero_outs = [], [], [], []
    for alloc in nc.m.functions[0].allocations:
        if not isinstance(alloc, mybir.MemoryLocationSet):
            continue
        name = alloc.memorylocations[0].name
        if alloc.kind == "ExternalInput":
            if name != partition_name:
                in_names.append(name)
        elif alloc.kind == "ExternalOutput":
            dt_np = mybir.dt.np(alloc.dtype)
            shape = tuple(alloc.tensor_shape)
            out_avals.append(jax.core.ShapedArray(shape, dt_np))
            out_names.append(name)
            zero_outs.append(_np.zeros(shape, dt_np))

    n_params = len(in_names)
    n_outs = len(out_names)
    all_in_names = list(in_names) + list(out_names)
    if partition_name is not None:
        all_in_names.append(partition_name)
    donate = tuple(range(n_params, n_params + n_outs))

    def _body(*args):
        operands = list(args)
        if partition_name is not None:
            operands.append(bass2jax.partition_id_tensor())
        outs = bass2jax._bass_exec_p.bind(
            *operands,
            out_avals=tuple(out_avals),
            in_names=tuple(all_in_names),
            out_names=tuple(out_names),
            lowering_input_output_aliases=(),
            sim_require_finite=True,
            sim_require_nnan=True,
            nc=nc,
        )
        return tuple(outs)

    devices = jax.devices()[:B]
    mesh = Mesh(np.asarray(devices), ("core",))
    in_specs = (PartitionSpec("core"),) * (n_params + n_outs)
    out_specs = (PartitionSpec("core"),) * n_outs
    sharded = jax.jit(
        shard_map(_body, mesh=mesh, in_specs=in_specs, out_specs=out_specs,
                  check_rep=False),
        donate_argnums=donate, keep_unused=True,
    )
    _CACHE[("mesh", repeat, VARIANT)] = mesh
    _CACHE[("jit", repeat, VARIANT)] = sharded

    def run(in_maps):
        concat_in = [
            np.concatenate([np.asarray(in_maps[c][k]) for c in range(B)], axis=0)
            for k in in_names
        ]
        concat_zeros = [np.zeros((B * z.shape[0], *z.shape[1:]), z.dtype)
                        for z in zero_outs]
        out_arrs = sharded(*concat_in, *concat_zeros)
        return out_arrs, out_names, out_avals

    _CACHE[key] = run
    return run


def run_fast(in_maps, repeat=1):
    """Execute via the cached jitted callable; returns per-core dict list."""
    run = _get_executor(repeat)
    out_arrs, out_names, out_avals = run(in_maps)
    return [
        {name: np.asarray(out_arrs[i]).reshape(B, *out_avals[i].shape)[c]
         for i, name in enumerate(out_names)}
        for c in range(B)
    ]


def bench_wall(in_maps, repeat, n_iter):
    """Dispatch n_iter executions of the repeat-R NEFF with device-resident
    inputs and pre-staged donated zero buffers; return total wall seconds."""
    import time as _time
    import jax
    from jax.sharding import NamedSharding, PartitionSpec

    _get_executor(repeat)  # ensure built
    nc = _get_nc(repeat)
    from concourse import mybir
    partition_name = nc.partition_id_tensor.name if nc.partition_id_tensor else None
    in_names, out_shapes = [], []
    for alloc in nc.m.functions[0].allocations:
        if not isinstance(alloc, mybir.MemoryLocationSet):
            continue
        name = alloc.memorylocations[0].name
        if alloc.kind == "ExternalInput" and name != partition_name:
            in_names.append(name)
        elif alloc.kind == "ExternalOutput":
            out_shapes.append((tuple(alloc.tensor_shape), mybir.dt.np(alloc.dtype)))

    key = ("bench_in", repeat, VARIANT)
    if key not in _CACHE:
        mesh = _CACHE[("mesh", repeat, VARIANT)]
        sh = NamedSharding(mesh, PartitionSpec("core"))
        dev_in = [
            jax.device_put(
                np.concatenate([np.asarray(in_maps[c][k]) for c in range(B)], 0), sh)
            for k in in_names
        ]
        _CACHE[key] = (dev_in, sh)
    dev_in, sh = _CACHE[key]

    sharded = _CACHE[("jit", repeat, VARIANT)]
    zero_sets = []
    for _ in range(n_iter):
        zs = [jax.device_put(np.zeros((B * s[0], *s[1:]), dt), sh)
              for (s, dt) in out_shapes]
        zero_sets.append(zs)
    for zs in zero_sets:
        for z in zs:
            z.block_until_ready()

    outs = []
    t0 = _time.perf_counter()
    for it in range(n_iter):
        outs.append(sharded(*dev_in, *zero_sets[it]))
    for o in outs[-1]:
        o.block_until_ready()
    t1 = _time.perf_counter()
    return t1 - t0


def bench_alternating(in_maps, r1, r2, n_pairs):
    """Alternate single dispatches of the repeat-r1 and repeat-r2 NEFFs,
    blocking after each; per-iter ns = (median(w2) - median(w1)) /
    (r2 - r1). Alternation cancels slow host/RPC drift; medians kill
    spikes."""
    import time as _time
    import jax
    from jax.sharding import NamedSharding, PartitionSpec

    from concourse import mybir

    def setup(repeat):
        _get_executor(repeat)
        nc = _get_nc(repeat)
        pn = nc.partition_id_tensor.name if nc.partition_id_tensor else None
        in_names, out_shapes = [], []
        for alloc in nc.m.functions[0].allocations:
            if not isinstance(alloc, mybir.MemoryLocationSet):
                continue
            name = alloc.memorylocations[0].name
            if alloc.kind == "ExternalInput" and name != pn:
                in_names.append(name)
            elif alloc.kind == "ExternalOutput":
                out_shapes.append(
                    (tuple(alloc.tensor_shape), mybir.dt.np(alloc.dtype)))
        mesh = _CACHE[("mesh", repeat, VARIANT)]
        sh = NamedSharding(mesh, PartitionSpec("core"))
        dev_in = [
            jax.device_put(
                np.concatenate([np.asarray(in_maps[c][k]) for c in range(B)], 0),
                sh)
            for k in in_names
        ]
        sharded = _CACHE[("jit", repeat, VARIANT)]
        return sharded, dev_in, out_shapes, sh

    s1, din1, osh1, sh1 = setup(r1)
    s2, din2, osh2, sh2 = setup(r2)

    def zeros_for(osh, sh):
        zs = [jax.device_put(np.zeros((B * s[0], *s[1:]), dt), sh)
              for (s, dt) in osh]
        for z in zs:
            z.block_until_ready()
        return zs

    def one(sharded, dev_in, zs):
        t0 = _time.perf_counter()
        outs = sharded(*dev_in, *zs)
        for o in outs:
            o.block_until_ready()
        return _time.perf_counter() - t0

    # warm both
    one(s1, din1, zeros_for(osh1, sh1))
    one(s2, din2, zeros_for(osh2, sh2))
    w1, w2 = [], []
    for _ in range(n_pairs):
        z1 = zeros_for(osh1, sh1)
        z2 = zeros_for(osh2, sh2)
        w1.append(one(s1, din1, z1))
        w2.append(one(s2, din2, z2))
    w1 = np.asarray(w1)
    w2 = np.asarray(w2)
    med = (np.median(w2) - np.median(w1)) / (r2 - r1) * 1e9
    lo = (np.percentile(w2, 25) - np.percentile(w1, 75)) / (r2 - r1) * 1e9
    hi = (np.percentile(w2, 75) - np.percentile(w1, 25)) / (r2 - r1) * 1e9
    return med, lo, hi


def make_in_maps(x, y, h, w, conv_q_w, bn_q_g, bn_q_b,
                 conv_k_w, bn_k_g, bn_k_b, conv_v_w, bn_v_g, bn_v_b,
                 Wq, Wk, Wv, Wo, bo):
    assert int(h) == IMG and int(w) == IMG
    x = np.asarray(x, np.float32)
    y = np.asarray(y, np.float32)
    wmap = _prep_weights(conv_q_w, conv_k_w, conv_v_w, Wq, Wk, Wv, Wo,
                         bn_q_g, bn_q_b, bn_k_g, bn_k_b, bn_v_g, bn_v_b, bo)
    def pad_t(a):
        # [B, L, C] -> [B, C, 34*34] with zero border baked in
        at = np.transpose(a, (0, 2, 1)).reshape(B, C, IMG, IMG)
        ap = np.zeros((B, C, PAD, PAD), np.float32)
        ap[:, :, 1:33, 1:33] = at
        return ap.reshape(B, C, PAD * PAD)

    xT = pad_t(x)
    yT = pad_t(y)
    return [dict(wmap, xt=xT[b], yt=yT[b]) for b in range(B)]


def kernel(**inputs):
    in_maps = make_in_maps(**inputs)
    res = run_fast(in_maps)
    outs = [res[b]["out"] for b in range(B)]  # each [C, L]
    return np.ascontiguousarray(
        np.stack(outs, axis=0).transpose(0, 2, 1)).astype(np.float32)
